# revision 33
# baseline (speedup 1.0000x reference)
"""Mixtral sparse-MoE block with per-expert LoRA adapters on 8 Trainium2 cores.

Problem shapes: B=2, S=1024, H=2048, F=7168, E=8, R=32, top-K=2.
T = B*S = 2048 tokens.

Sharding: tensor-parallel over the FFN dim F. Core c owns rows
[c*896:(c+1)*896] of W1/W3 (and the matching B1/B3 LoRA rows) and the same
columns of W2/A2. Everything after the silu is linear in
x2s = silu(x1)*x3*rw, so each core emits an exact partial [H, T] output over
its F-shard (bf16) and the host sums the 8 partials in fp32.

Work split (device vs host):
- Host: gating (softmax + top-2), the tiny per-expert LoRA down-projections
  a1/a3 = x @ A{1,3}T masked per slot (m1/m3 uploads, fp8e4), and the final
  LoRA up-projection lora2 = B2 @ sum_cores(m2) (one small GEMM).
- Device (per core): base1/base3 = x @ W{1,3}[shard].T, the per-slot LoRA
  up-projections lora1/3 = B{1,3}[shard] @ m{1,3} accumulated in PSUM,
  silu/mul/scale chain, a2 = A2[shard] @ x2s per slot (m2, returned to
  host), and the down-projection on the slot-summed activations.

Perf structure (measured on trn2 via NTFF hardware traces):
- PE-bound: a dense bf16 MM stream runs ~216ns per K=128/N=512 matmul at
  2.4GHz. Reductions vs the v1 kernel:
  * w1/w3 are streamed F-CHUNK-major ([NF,128,NH,128] host packs): block-0
    phase A consumes weights in DMA arrival order, so the PE starts ~5us
    earlier and the ~7us of warmup stalls (waiting for h-major pair DMAs)
    disappear.
  * Tokens are sorted by slot-0 expert, then inside each block by
    (slot0-chunk, slot1-chunk). Each block decomposes into contiguous
    column segments per slot that touch a single er-chunk, so the LoRA
    up-projections and a2 contract every token column against only ITS
    er-chunk: ~half the LoRA matmul columns vs dense. (fp8 DoubleRow was
    tried for the dense case and measured at only ~6% over two bf16
    matmuls — column splitting beats it and needs no perf mode.)
  * m1/m3/b1/b3 are host-quantized fp8e4 with power-of-2 scales (S_M=16,
    S_B=1024); the product scale 1/16384 is folded into the silu's input
    scale and the host-prescaled rw tensor — zero dequant ops on device.
  * outT is written bf16 (halves the output drain) and output DMAs
    alternate between the sync/scalar HWDGE rings.
  * Phase-A PSUM->SBUF copies run on the DVE, not the scalar engine: the
    scalar engine's stream is clogged by DMA-issue instructions during
    warmup (each blocks on HWDGE queue space), which held PSUM banks
    hostage and stalled phase A for ~17us.
- The block loop is software-pipelined: block b+1's phase-A groups are
  EMITTED between block b's phase-B iterations. Phase B is DVE-bound
  (~2.2us of add/silu/mul chain per f-iteration), and the PE executes in
  emission order, so each interposed phase-A group gives the DVE a ~7us
  matmul window to drain its backlog. The last block (no next A) instead
  weaves in the held-back phase-C groups of block NT-2.
- Every dma_start costs ~0.6us of ISSUE time on its HWDGE ring regardless
  of size; streamed tensors are host-packed so one DMA feeds multiple SBUF
  chunk views. Only sync/scalar HWDGE rings are used.
- Outputs are unmasked a2 partials; the (elementwise) expert mask commutes
  with the cross-core sum and is applied on host before the lora2 GEMM.
"""

import sys
from contextlib import ExitStack

import numpy as np

try:
    import concourse.bass as bass  # noqa: F401
except ImportError:
    sys.path.insert(0, "/opt/trn_rl_repo")

import ml_dtypes

import concourse.bass as bass
import concourse.mybir as mybir
import concourse.tile as tile
from concourse import bacc
from concourse.bass_utils import run_bass_kernel_spmd

BF16 = mybir.dt.bfloat16
F8E4 = mybir.dt.float8e4
F32 = mybir.dt.float32
NPBF16 = ml_dtypes.bfloat16
NPF8E4 = ml_dtypes.float8_e4m3

B, S, H, F, E, R, K = 2, 1024, 2048, 7168, 8, 32, 2
T = B * S                      # 2048 tokens
ER = E * R                     # 256
NCORES = 8
FS = F // NCORES               # 896 per-core F shard
NH = H // 128                  # 16 h-chunks
NF = FS // 128                 # 7 f-chunks (per core)
NER = ER // 128                # 2 er-chunks
TBLK = 512
NT = T // TBLK                 # 4 token blocks

S_M = 16.0                     # fp8 scale on m1/m3 (a-values, sigma ~0.9)
S_B = 1024.0                   # fp8 scale on b1/b3 (weights, sigma ~0.02)
SCALE = S_M * S_B              # lora PSUM scale; folded into silu + rw


def build_nc(repeat=None, spec=None):
    """Build the per-core Bass module.

    spec: per-block tuple of active er-chunks for slot 0 (from the host's
    exact-expert token sort); None means dense (0, 1) everywhere.
    """
    nc = bacc.Bacc(None)

    # x is host-packed in h-chunk pairs [NH//2, 128, 2, T]; one DMA feeds
    # two SBUF chunk-views.
    xT = nc.declare_dram_parameter("xT", [NH // 2, 128, 2, T], BF16, isOutput=False)
    # w1/w3 are host-packed F-CHUNK-major: [NF, 128, NH, 128] — w1t[f][k,hc,m]
    # = W1[shard_f*128+m, hc*128+k]. One DMA per f-chunk means block-0's
    # phase A consumes weights in arrival order (A(f) needs only chunk f).
    w1t = nc.declare_dram_parameter("w1t", [NF, 128, NH, 128], BF16, isOutput=False)
    w3t = nc.declare_dram_parameter("w3t", [NF, 128, NH, 128], BF16, isOutput=False)
    w2t = nc.declare_dram_parameter("w2t", [NF, 128, H], BF16, isOutput=False)
    # fp8 LoRA operands: m{1,3} [K, 128, NER, T] (x S_M), b{1,3} [128, NER, FS]
    # (x S_B). Layout matches DoubleRow's [Ki, Ko=2, dim] AP: partition k is
    # er-row c*128+k of chunk c.
    m1t = nc.declare_dram_parameter("m1t", [K, 128, NER, T], F8E4, isOutput=False)
    m3t = nc.declare_dram_parameter("m3t", [K, 128, NER, T], F8E4, isOutput=False)
    b1t = nc.declare_dram_parameter("b1t", [128, NER, FS], F8E4, isOutput=False)
    b3t = nc.declare_dram_parameter("b3t", [128, NER, FS], F8E4, isOutput=False)
    a2t = nc.declare_dram_parameter("a2t", [NF, 128, ER], BF16, isOutput=False)
    rwr = nc.declare_dram_parameter("rwr", [K, 1, T], BF16, isOutput=False)
    outT = nc.declare_dram_parameter("outT", [NH, 128, T], BF16, isOutput=True)
    m2o = nc.declare_dram_parameter("m2o", [K, NER, 128, T], BF16, isOutput=True)

    with tile.TileContext(nc) as tc, ExitStack() as ctx:
        resw = ctx.enter_context(tc.tile_pool(name="resw", bufs=1))
        xsp = ctx.enter_context(tc.tile_pool(name="xsp", bufs=2))
        actp = ctx.enter_context(tc.tile_pool(name="actp", bufs=1))
        mp_ = ctx.enter_context(tc.tile_pool(name="mp", bufs=2))
        trans = ctx.enter_context(tc.tile_pool(name="trans", bufs=3))
        outp = ctx.enter_context(tc.tile_pool(name="outp", bufs=4))
        # PSUM partition: phase A gets 4 banks (2 tags x 2 bufs), phase B's
        # short LoRA groups 2 banks, a2 + down-proj share 2 banks.
        psp = ctx.enter_context(tc.tile_pool(name="psp", bufs=2, space="PSUM"))
        pspB = ctx.enter_context(tc.tile_pool(name="pspB", bufs=1, space="PSUM"))
        pspD = ctx.enter_context(tc.tile_pool(name="pspD", bufs=2, space="PSUM"))

        loop_cm = tc.For_i(0, repeat, 1) if repeat is not None else None
        if loop_cm is not None:
            loop_cm.__enter__()

        # ---- per-block input streamers ----
        def load_block_inputs(tb, xs=None):
            tsl = slice(tb * TBLK, (tb + 1) * TBLK)
            if xs is None:
                xs = []
                for hp in range(NH // 2):
                    xt_ = xsp.tile([128, 2 * TBLK], BF16, name=f"x{hp}",
                                   tag=f"x{hp}")
                    nc.sync.dma_start(out=xt_, in_=xT[hp][:, :, tsl])
                    xs.append(xt_[:, 0:TBLK])
                    xs.append(xt_[:, TBLK:2 * TBLK])
            m1, m3 = [None] * K, [None] * K
            for k in range(K):
                m1_ = mp_.tile([128, NER, TBLK], F8E4, name=f"m1_{k}",
                               tag=f"m1_{k}")
                nc.sync.dma_start(out=m1_, in_=m1t[k][:, :, tsl])
                m1[k] = m1_
                m3_ = mp_.tile([128, NER, TBLK], F8E4, name=f"m3_{k}",
                               tag=f"m3_{k}")
                nc.scalar.dma_start(out=m3_, in_=m3t[k][:, :, tsl])
                m3[k] = m3_
            rws = []
            for k in range(K):
                r_ = mp_.tile([128, TBLK], BF16, name=f"rw{k}", tag=f"rw{k}")
                nc.sync.dma_start(out=r_, in_=rwr[k][:, tsl].to_broadcast([128, TBLK]))
                rws.append(r_)
            return xs, rws, m1, m3

        # ---- resident weights, emitted in CONSUMPTION order so the two
        # ~170GB/s HWDGE rings deliver each tensor just before phase A of
        # block 0 needs it: f0 (split in h-halves for the earliest first
        # matmul) interleaved with the first x pairs, then f1..f6, then the
        # lora/m inputs, then w2/a2 (needed ~60us in). ----
        w1f, w3f = [None] * NF, [None] * NF
        NQ = NH // 4
        w1f0, w3f0 = [None] * 4, [None] * 4   # h-quarters of f-chunk 0

        def load_wf0_quarter(i):
            hs = slice(i * NQ, (i + 1) * NQ)
            t1 = resw.tile([128, NQ, 128], BF16, name=f"w1f0{i}",
                           tag=f"w1f0{i}")
            nc.sync.dma_start(out=t1, in_=w1t[0][:, hs, :])
            w1f0[i] = t1
            t3 = resw.tile([128, NQ, 128], BF16, name=f"w3f0{i}",
                           tag=f"w3f0{i}")
            nc.scalar.dma_start(out=t3, in_=w3t[0][:, hs, :])
            w3f0[i] = t3

        def load_wf(f):
            t1 = resw.tile([128, NH, 128], BF16, name=f"w1f{f}", tag=f"w1f{f}")
            nc.sync.dma_start(out=t1, in_=w1t[f])
            w1f[f] = t1
            t3 = resw.tile([128, NH, 128], BF16, name=f"w3f{f}", tag=f"w3f{f}")
            nc.scalar.dma_start(out=t3, in_=w3t[f])
            w3f[f] = t3

        def w1v(f, h):
            if f == 0:
                return w1f0[h // NQ][:, h % NQ, :]
            return w1f[f][:, h, :]

        def w3v(f, h):
            if f == 0:
                return w3f0[h // NQ][:, h % NQ, :]
            return w3f[f][:, h, :]

        xs0 = []

        def load_x0_single(h):
            # first h-pair loaded as two singles (one per ring) so the very
            # first matmul waits on 128KB, not a 256KB pair
            xt_ = xsp.tile([128, TBLK], BF16, name=f"x0s{h}", tag=f"x0s{h}")
            eng = nc.sync if h == 0 else nc.scalar
            eng.dma_start(out=xt_, in_=xT[0][:, h, 0:TBLK])
            xs0.append(xt_)

        def load_x0_pair(hp):
            xt_ = xsp.tile([128, 2 * TBLK], BF16, name=f"x{hp}", tag=f"x{hp}")
            xeng = nc.sync if hp % 2 == 0 else nc.scalar
            xeng.dma_start(out=xt_, in_=xT[hp][:, :, 0:TBLK])
            xs0.append(xt_[:, 0:TBLK])
            xs0.append(xt_[:, TBLK:2 * TBLK])

        load_wf0_quarter(0)
        load_x0_single(0); load_x0_single(1)
        load_x0_pair(1)
        load_wf0_quarter(1)
        load_x0_pair(2); load_x0_pair(3)
        load_wf0_quarter(2)
        load_x0_pair(4); load_x0_pair(5)
        load_wf0_quarter(3)
        load_x0_pair(6); load_x0_pair(7)
        for f in range(1, NF):
            load_wf(f)
        b1s = resw.tile([128, NER, FS], F8E4, name="b1s", tag="b1s")
        nc.sync.dma_start(out=b1s, in_=b1t[:, :, :])
        b3s = resw.tile([128, NER, FS], F8E4, name="b3s", tag="b3s")
        nc.scalar.dma_start(out=b3s, in_=b3t[:, :, :])
        pre0 = load_block_inputs(0, xs0)
        w2s, a2s = [], []
        for f in range(NF):
            eng = nc.sync if f % 2 == 0 else nc.scalar
            t_ = resw.tile([128, H], BF16, name=f"w2s{f}", tag=f"w2s{f}")
            eng.dma_start(out=t_, in_=w2t[f])
            w2s.append(t_)
        for f in range(NF):
            eng = nc.scalar if f % 2 == 0 else nc.sync
            t_ = resw.tile([128, ER], BF16, name=f"a2s{f}", tag=f"a2s{f}")
            eng.dma_start(out=t_, in_=a2t[f])
            a2s.append(t_)

        # ---- phase emitters (software-pipelined across blocks below) ----
        def emit_A_group(xs, f, base1, base3):
            """One f-chunk of base1/base3 = W1/W3 @ x (PE-dense, no deps).
            The PSUM->SBUF copies scale by SCALE so phase B's adds work in
            the fp8-product scale with zero extra ops."""
            ps1 = psp.tile([128, TBLK], F32, name="ps1", tag="pA")
            ps3 = psp.tile([128, TBLK], F32, name="ps3", tag="pB")
            for h in range(NH):
                nc.tensor.matmul(ps1, w1v(f, h), xs[h], start=(h == 0), stop=(h == NH - 1))
                nc.tensor.matmul(ps3, w3v(f, h), xs[h], start=(h == 0), stop=(h == NH - 1))
            # copies ride the DVE: the scalar engine's stream is clogged by
            # DMA-issue instructions early on (queue-full waits), and a
            # scalar copy here would delay the PSUM bank release that gates
            # the next A group's leader matmul.
            b1_ = actp.tile([128, TBLK], BF16, name=f"b1_{f}", tag=f"b1_{f}")
            nc.vector.tensor_scalar_mul(b1_, ps1, SCALE)
            base1[f] = b1_
            b3_ = actp.tile([128, TBLK], BF16, name=f"b3_{f}", tag=f"b3_{f}")
            nc.vector.tensor_scalar_mul(b3_, ps3, SCALE)
            base3[f] = b3_

        def emit_A(xs):
            base1, base3 = [None] * NF, [None] * NF
            for f in range(NF):
                emit_A_group(xs, f, base1, base3)
            return base1, base3

        def emit_xsum(f, x2s, xsum):
            xs_ = actp.tile([128, TBLK], BF16, name=f"xsum{f}",
                            tag=f"xsum{f}")
            # alternate gpsimd/DVE: gpsimd's ~2.3us per add serializes all
            # seven xsums, and xsum[6] gates the block's first C group
            eng = nc.gpsimd if f % 2 == 0 else nc.vector
            eng.tensor_add(xs_, x2s[0][f], x2s[1][f])
            xsum[f] = xs_

        def segs_of(tb, k):
            """Column segments (er_chunk, c0, c1) covering the block. The
            host sub-sorts tokens inside each block by (slot0-chunk,
            slot1-chunk), so every token column is contracted against ONLY
            its own er-chunk — the LoRA matmul work per column halves vs
            contracting both chunks everywhere."""
            if spec is None:
                return ((0, 0, TBLK), (1, 0, TBLK))
            return spec[tb][k]

        def emit_B_f(tb, k, f, base1, base3, rws, m1, m3, x2s, xsum,
                     do_xsum=True, altps=False):
            """LoRA up-proj + silu/mul chain for one (slot, f-chunk).
            altps (last block): alternate the PSUM tags between pspB's
            qA/qB and psp's idle pA/pB so each tag is reused every OTHER
            iteration — the DVE chain then never gates the leader matmul."""
            segs = segs_of(tb, k)
            fsl = slice(f * 128, (f + 1) * 128)
            if altps and f % 2 == 1:
                psA = psp.tile([128, TBLK], F32, name="psA", tag="pA")
                psB = psp.tile([128, TBLK], F32, name="psB", tag="pB")
            else:
                psA = pspB.tile([128, TBLK], F32, name="psA", tag="qA")
                psB = pspB.tile([128, TBLK], F32, name="psB", tag="qB")
            for (er, c0, c1) in segs:
                nc.tensor.matmul(psA[:, c0:c1], b1s[:, er, fsl],
                                 m1[k][:, er, c0:c1], start=True, stop=True)
            for (er, c0, c1) in segs:
                nc.tensor.matmul(psB[:, c0:c1], b3s[:, er, fsl],
                                 m3[k][:, er, c0:c1], start=True, stop=True)
            t1_ = trans.tile([128, TBLK], BF16, name="t1", tag="t1")
            nc.vector.tensor_add(t1_, psA, base1[f])
            sl_ = trans.tile([128, TBLK], BF16, name="sl", tag="sl")
            nc.scalar.activation(sl_, t1_, mybir.ActivationFunctionType.Silu,
                                 scale=1.0 / SCALE)
            t3_ = trans.tile([128, TBLK], BF16, name="t3", tag="t3")
            nc.vector.tensor_add(t3_, psB, base3[f])
            x3s_ = trans.tile([128, TBLK], BF16, name="x3s", tag="x3s")
            nc.vector.tensor_mul(x3s_, t3_, rws[k])
            x2_ = actp.tile([128, TBLK], BF16, name=f"x2_{k}{f}",
                            tag=f"x2_{k}{f}")
            nc.vector.tensor_mul(x2_, sl_, x3s_)
            x2s[k][f] = x2_
            if k == K - 1 and do_xsum:
                emit_xsum(f, x2s, xsum)

        def emit_a2(tb, k, x2s):
            t0 = tb * TBLK
            segs = segs_of(tb, k)
            for er in range(NER):
                ranges = [(c0, c1) for (e, c0, c1) in segs if e == er]
                if not ranges:
                    continue
                ers = slice(er * 128, (er + 1) * 128)
                psa2 = pspD.tile([128, TBLK], F32, name="psa2", tag="pD")
                for (c0, c1) in ranges:
                    for f in range(NF):
                        nc.tensor.matmul(psa2[:, c0:c1], a2s[f][:, ers],
                                         x2s[k][f][:, c0:c1],
                                         start=(f == 0), stop=(f == NF - 1))
                m2_ = actp.tile([128, TBLK], BF16, name=f"m2_{k}{er}",
                                tag=f"m2_{k}{er}")
                eng = nc.scalar if (k + er) % 2 == 0 else nc.sync
                # copy/DMA only the covered ranges: the uncovered psa2
                # columns are stale PSUM (m2o is zero-initialized and the
                # host masks per-token, so untouched regions contribute 0).
                for (c0, c1) in ranges:
                    nc.scalar.copy(m2_[:, c0:c1], psa2[:, c0:c1])
                    eng.dma_start(out=m2o[k][er][:, t0 + c0:t0 + c1],
                                  in_=m2_[:, c0:c1])

        def emit_C_group(tb, xsum, h, lastblk=False):
            tsl = slice(tb * TBLK, (tb + 1) * TBLK)
            hsl = slice(h * 128, (h + 1) * 128)
            if lastblk:
                # pA/pB belong to the last block's Bx iterations there
                psD = pspD.tile([128, TBLK], F32, name="psD", tag="pD")
            else:
                # ride the phase-A banks (idle once A'(5)/A'(6) have been
                # copied out): double-buffered leaders instead of sharing
                # pD's 2 banks with a2
                psD = psp.tile([128, TBLK], F32, name="psD",
                               tag=("pA" if h % 2 == 0 else "pB"))
            for f in range(NF):
                nc.tensor.matmul(psD, w2s[f][:, hsl], xsum[f],
                                 start=(f == 0), stop=(f == NF - 1))
            o_ = outp.tile([128, TBLK], BF16, name="osb", tag="osb")
            nc.scalar.copy(o_, psD)
            eng = nc.sync if h % 2 == 0 else nc.scalar
            eng.dma_start(out=outT[h][:, tsl], in_=o_)

        def emit_C_group_split(tb, xsum, h):
            """Very last C group, column-split with TWO half-bank PSUM
            tiles so half-1's copy+DMA overlap half-2's matmuls (a single
            tile serializes on Tile's per-tile read/write tracking)."""
            t0 = tb * TBLK
            hsl = slice(h * 128, (h + 1) * 128)
            HB = TBLK // 2
            for i, c0 in enumerate((0, HB)):
                psD = pspD.tile([128, HB], F32, name=f"psDs{i}", tag="pD")
                for f in range(NF):
                    nc.tensor.matmul(psD, w2s[f][:, hsl],
                                     xsum[f][:, c0:c0 + HB],
                                     start=(f == 0), stop=(f == NF - 1))
                o_ = outp.tile([128, HB], BF16, name="osbs", tag="osb")
                nc.scalar.copy(o_, psD)
                eng = nc.sync if i == 0 else nc.scalar
                eng.dma_start(out=outT[h][:, t0 + c0:t0 + c0 + HB], in_=o_)

        def emit_C(tb, xsum, last=False):
            for h in range(NH):
                if last and h == NH - 1:
                    emit_C_group_split(tb, xsum, h)
                else:
                    emit_C_group(tb, xsum, h, lastblk=last)

        # ---- software pipeline: next block's phase-A groups are woven
        # BETWEEN this block's phase-B iterations (PE executes in emission
        # order, so independent work must be emitted before gated work).
        # Phase B is Vector-throughput-bound (~2.2us of DVE chain per
        # f-iteration); each interposed A group gives the DVE ~7us of
        # matmul cover to drain its chain backlog, so the B-group PSUM
        # leaders never wait on bank release. ----
        xs, rws, m1, m3 = pre0
        base1, base3 = emit_A(xs)
        heldC = None   # xsum of block NT-2, its C woven into the last block
        for tb in range(NT):
            x2s = [[None] * NF for _ in range(K)]
            xsum = [None] * NF
            Bf = lambda k, f: emit_B_f(tb, k, f, base1, base3, rws, m1, m3,
                                       x2s, xsum)
            if tb + 1 < NT:
                xsn, rwsn, m1n, m3n = load_block_inputs(tb + 1)
                b1n, b3n = [None] * NF, [None] * NF
                A = lambda f: emit_A_group(xsn, f, b1n, b3n)
                Bf(0, 0); Bf(0, 1)
                Bf(0, 2); Bf(0, 3); A(0)
                Bf(0, 4); Bf(0, 5); A(1)
                Bf(0, 6); Bf(1, 0); A(2)
                Bf(1, 1); Bf(1, 2); A(3)
                Bf(1, 3); Bf(1, 4); emit_a2(tb, 0, x2s)
                Bf(1, 5); Bf(1, 6); A(4)
                emit_a2(tb, 1, x2s)
                A(5); A(6)
                xs, rws, m1, m3 = xsn, rwsn, m1n, m3n
                base1, base3 = b1n, b3n
                if tb == NT - 2:
                    heldC = xsum      # defer C(NT-2) into the last block
                else:
                    emit_C(tb, xsum)
            else:
                # last block has no next-A cover; weave the held-back
                # C(NT-2) groups among the k=0 iterations instead. All held
                # groups must be emitted before B(1,0) writes xsum (the
                # single-buffered xsum tags roll over to this block there).
                hq = list(range(NH))
                C2 = lambda n: [emit_C_group(tb - 1, heldC, hq.pop(0),
                                             lastblk=True)
                                for _ in range(n)]
                Bx = lambda k, f: emit_B_f(tb, k, f, base1, base3, rws, m1,
                                           m3, x2s, xsum, do_xsum=False,
                                           altps=True)
                Bx(0, 0); Bx(1, 0); C2(3)
                Bx(0, 1); Bx(1, 1); C2(3)
                Bx(0, 2); Bx(1, 2); C2(2)
                Bx(0, 3); Bx(1, 3); C2(2)
                Bx(0, 4); Bx(1, 4); C2(2)
                Bx(0, 5); Bx(1, 5); C2(2)
                Bx(0, 6); Bx(1, 6); C2(2)
                for f in range(NF):
                    emit_xsum(f, x2s, xsum)
                emit_a2(tb, 0, x2s)
                emit_a2(tb, 1, x2s)
                emit_C(tb, xsum, last=True)

        if loop_cm is not None:
            loop_cm.__exit__(None, None, None)

    nc.finalize()
    return nc


def _q8(a, scale):
    return np.clip(a * scale, -240.0, 240.0).astype(NPF8E4)


def prepare_inputs(hidden_states, Wg, W1, W2, W3, A1, B1, A2, B2, A3, B3):
    """Host preprocessing: routing + per-core weight slicing/casting."""
    hidden_states, Wg, W1, W2, W3, A1, B1, A2, B2, A3, B3 = (
        np.asarray(a, dtype=np.float32)
        for a in (hidden_states, Wg, W1, W2, W3, A1, B1, A2, B2, A3, B3))
    x = np.ascontiguousarray(hidden_states.reshape(T, H))

    logits = x @ Wg.T.astype(np.float32)
    m = logits.max(-1, keepdims=True)
    p = np.exp(logits - m, dtype=np.float32)
    p /= p.sum(-1, keepdims=True)
    sel = np.argsort(-p, axis=-1, kind="stable")[:, :K]      # [T, K]
    rw = np.take_along_axis(p, sel, axis=1)
    rw = (rw / rw.sum(-1, keepdims=True)).astype(np.float32)  # [T, K]

    # Sort tokens by slot-0 EXPERT (block composition), then inside each
    # block by (slot0-chunk, slot1-chunk). Each block then decomposes into
    # a few contiguous column segments per slot, each touching a single
    # er-chunk — the device contracts every token column against only ITS
    # chunk instead of both.
    GE = E // NER                         # experts per er-chunk
    perm = np.argsort(sel[:, 0], kind="stable")
    for b in range(NT):
        idx = perm[b * TBLK:(b + 1) * TBLK]
        key = (sel[idx, 0] // GE) * NER + (sel[idx, 1] // GE)
        perm[b * TBLK:(b + 1) * TBLK] = idx[np.argsort(key, kind="stable")]
    x = np.ascontiguousarray(x[perm])
    sel = sel[perm]
    rw = np.ascontiguousarray(rw[perm])

    spec = []
    for b in range(NT):
        per_slot = []
        for k in range(K):
            ch = sel[b * TBLK:(b + 1) * TBLK, k] // GE
            segs, start = [], 0
            for i in range(1, TBLK + 1):
                if i == TBLK or ch[i] != ch[i - 1]:
                    segs.append((int(ch[start]), start, i))
                    start = i
            per_slot.append(tuple(segs))
        spec.append(tuple(per_slot))
    spec = tuple(spec)

    xT_np = np.ascontiguousarray(
        x.T.reshape(NH // 2, 2, 128, T).transpose(0, 2, 1, 3)
    ).astype(NPBF16)                                  # [NH//2, 128, 2, T]

    # per-slot one-hot masks over the (e, r) axis, transposed to [ER, T];
    # applied HOST-side to the returned a2 (masking is elementwise, so it
    # commutes with the cross-core partial sum)
    masks = np.zeros((K, ER, T), dtype=np.float32)
    for k in range(K):
        onehot = np.zeros((T, E), np.float32)
        onehot[np.arange(T), sel[:, k]] = 1.0
        masks[k] = np.repeat(onehot, R, axis=1).T
    # rw is pre-divided by SCALE: the device's x3 path multiplies the
    # SCALE-scaled (base3 + lora3) PSUM values by it, landing on true scale.
    rwr_np = np.ascontiguousarray(rw.T / SCALE).reshape(K, 1, T).astype(NPBF16)

    # flattened LoRA tensors (full copies; small)
    A1f = A1.reshape(ER, H)                      # [er, H]
    A3f = A3.reshape(ER, H)
    B2f = B2.transpose(0, 2, 1).reshape(ER, H)   # [er, H]

    # per-slot masked LoRA down-projections, computed host-side in fp32,
    # quantized to fp8e4 (x S_M) in the DoubleRow [Ki=128, Ko=NER, T] layout
    a1_all = x @ A1f.T.astype(np.float32)        # [T, ER]
    a3_all = x @ A3f.T.astype(np.float32)
    m1t_np = np.zeros((K, ER, T), dtype=NPF8E4)
    m3t_np = np.zeros((K, ER, T), dtype=NPF8E4)
    for k in range(K):
        mx = np.repeat(
            np.eye(E, dtype=np.float32)[sel[:, k]], R, axis=1)   # [T, ER]
        m1t_np[k] = _q8((a1_all * mx).T, S_M)
        m3t_np[k] = _q8((a3_all * mx).T, S_M)
    m1t_np = np.ascontiguousarray(
        m1t_np.reshape(K, NER, 128, T).transpose(0, 2, 1, 3))
    m3t_np = np.ascontiguousarray(
        m3t_np.reshape(K, NER, 128, T).transpose(0, 2, 1, 3))

    def pack_fmajor(wT):
        # [FS, H] -> [NF, 128, NH, 128]: [f, k, hc, m] = W[f*128+m, hc*128+k]
        return np.ascontiguousarray(
            wT.reshape(NF, 128, NH, 128).transpose(0, 3, 2, 1))

    in_maps = []
    for c in range(NCORES):
        fs = slice(c * FS, (c + 1) * FS)
        w1t_np = pack_fmajor(W1[fs]).astype(NPBF16)
        w3t_np = pack_fmajor(W3[fs]).astype(NPBF16)
        w2T = np.ascontiguousarray(W2[:, fs].T).astype(NPBF16)  # [FS, H]
        w2t_np = w2T.reshape(NF, 128, H)
        b1f = B1[:, fs, :].transpose(0, 2, 1).reshape(ER, FS)   # [er, f]
        b3f = B3[:, fs, :].transpose(0, 2, 1).reshape(ER, FS)
        b1t_np = np.ascontiguousarray(
            _q8(b1f, S_B).reshape(NER, 128, FS).transpose(1, 0, 2))
        b3t_np = np.ascontiguousarray(
            _q8(b3f, S_B).reshape(NER, 128, FS).transpose(1, 0, 2))
        a2f = A2[:, :, fs].reshape(ER, FS)                      # [er, f]
        a2t_np = np.ascontiguousarray(a2f.T).astype(NPBF16).reshape(NF, 128, ER)

        in_maps.append({
            "xT": xT_np, "w1t": w1t_np, "w3t": w3t_np, "w2t": w2t_np,
            "m1t": m1t_np, "m3t": m3t_np, "b1t": b1t_np, "b3t": b3t_np,
            "a2t": a2t_np,
            "rwr": rwr_np,
        })
    return in_maps, (B2f.astype(np.float32), masks, perm, spec)


_CACHED_NC = {}


def kernel(hidden_states, Wg, W1, W2, W3, A1, B1, A2, B2, A3, B3,
           _trace=False, _tmpdir=None):
    in_maps, (B2f, masks, perm, spec) = prepare_inputs(
        hidden_states, Wg, W1, W2, W3, A1, B1, A2, B2, A3, B3)
    if spec not in _CACHED_NC:
        _CACHED_NC[spec] = build_nc(spec=spec)
    nc = _CACHED_NC[spec]
    res = run_bass_kernel_spmd(nc, in_maps, list(range(NCORES)),
                               trace=_trace, tmpdir=_tmpdir)
    acc = np.zeros((NH, 128, T), np.float32)
    m2sum = np.zeros((K, ER, T), np.float32)
    for c in range(NCORES):
        acc += res.results[c]["outT"].astype(np.float32)
        m2sum += res.results[c]["m2o"].reshape(K, ER, T).astype(np.float32)
    out = acc.reshape(H, T)
    # host-side lora2: mask the (unmasked, core-summed) a2, then the final
    # LoRA up-projection is linear -> one small GEMM per slot
    for k in range(K):
        out += B2f.T @ (m2sum[k] * masks[k])
    outT_tok = out.T                       # [T, H], token-permuted order
    final = np.empty_like(outT_tok)
    final[perm] = outT_tok                 # undo the expert sort
    out = final.reshape(B, S, H)
    kernel.last_results = res
    return out


if __name__ == "__main__":
    nc = build_nc(spec=None)
    print("built ok")


# revision 36
# speedup vs baseline: 1.0020x; 1.0020x over previous
"""Mixtral sparse-MoE block with per-expert LoRA adapters on 8 Trainium2 cores.

Problem shapes: B=2, S=1024, H=2048, F=7168, E=8, R=32, top-K=2.
T = B*S = 2048 tokens.

Sharding: tensor-parallel over the FFN dim F. Core c owns rows
[c*896:(c+1)*896] of W1/W3 (and the matching B1/B3 LoRA rows) and the same
columns of W2/A2. Everything after the silu is linear in
x2s = silu(x1)*x3*rw, so each core emits an exact partial [H, T] output over
its F-shard (bf16) and the host sums the 8 partials in fp32.

Work split (device vs host):
- Host: gating (softmax + top-2), the tiny per-expert LoRA down-projections
  a1/a3 = x @ A{1,3}T masked per slot (m1/m3 uploads, fp8e4), and the final
  LoRA up-projection lora2 = B2 @ sum_cores(m2) (one small GEMM).
- Device (per core): base1/base3 = x @ W{1,3}[shard].T, the per-slot LoRA
  up-projections lora1/3 = B{1,3}[shard] @ m{1,3} accumulated in PSUM,
  silu/mul/scale chain, a2 = A2[shard] @ x2s per slot (m2, returned to
  host), and the down-projection on the slot-summed activations.

Perf structure (measured on trn2 via NTFF hardware traces):
- PE-bound: a dense bf16 MM stream runs ~216ns per K=128/N=512 matmul at
  2.4GHz. Reductions vs the v1 kernel:
  * w1/w3 are streamed F-CHUNK-major ([NF,128,NH,128] host packs): block-0
    phase A consumes weights in DMA arrival order, so the PE starts ~5us
    earlier and the ~7us of warmup stalls (waiting for h-major pair DMAs)
    disappear.
  * Tokens are sorted by slot-0 expert, then inside each block by
    (slot0-chunk, slot1-chunk). Each block decomposes into contiguous
    column segments per slot that touch a single er-chunk, so the LoRA
    up-projections and a2 contract every token column against only ITS
    er-chunk: ~half the LoRA matmul columns vs dense. (fp8 DoubleRow was
    tried for the dense case and measured at only ~6% over two bf16
    matmuls — column splitting beats it and needs no perf mode.)
  * m1/m3/b1/b3 are host-quantized fp8e4 with power-of-2 scales (S_M=16,
    S_B=1024); the product scale 1/16384 is folded into the silu's input
    scale and the host-prescaled rw tensor — zero dequant ops on device.
  * outT is written bf16 (halves the output drain) and output DMAs
    alternate between the sync/scalar HWDGE rings.
  * Phase-A PSUM->SBUF copies run on the DVE, not the scalar engine: the
    scalar engine's stream is clogged by DMA-issue instructions during
    warmup (each blocks on HWDGE queue space), which held PSUM banks
    hostage and stalled phase A for ~17us.
- The block loop is software-pipelined: block b+1's phase-A groups are
  EMITTED between block b's phase-B iterations. Phase B is DVE-bound
  (~2.2us of add/silu/mul chain per f-iteration), and the PE executes in
  emission order, so each interposed phase-A group gives the DVE a ~7us
  matmul window to drain its backlog. The last block (no next A) instead
  weaves in the held-back phase-C groups of block NT-2.
- Every dma_start costs ~0.6us of ISSUE time on its HWDGE ring regardless
  of size; streamed tensors are host-packed so one DMA feeds multiple SBUF
  chunk views. Only sync/scalar HWDGE rings are used.
- Outputs are unmasked a2 partials; the (elementwise) expert mask commutes
  with the cross-core sum and is applied on host before the lora2 GEMM.
"""

import sys
from contextlib import ExitStack

import numpy as np

try:
    import concourse.bass as bass  # noqa: F401
except ImportError:
    sys.path.insert(0, "/opt/trn_rl_repo")

import ml_dtypes

import concourse.bass as bass
import concourse.mybir as mybir
import concourse.tile as tile
from concourse import bacc
from concourse.bass_utils import run_bass_kernel_spmd

BF16 = mybir.dt.bfloat16
F8E4 = mybir.dt.float8e4
F32 = mybir.dt.float32
NPBF16 = ml_dtypes.bfloat16
NPF8E4 = ml_dtypes.float8_e4m3

B, S, H, F, E, R, K = 2, 1024, 2048, 7168, 8, 32, 2
T = B * S                      # 2048 tokens
ER = E * R                     # 256
NCORES = 8
FS = F // NCORES               # 896 per-core F shard
NH = H // 128                  # 16 h-chunks
NF = FS // 128                 # 7 f-chunks (per core)
NER = ER // 128                # 2 er-chunks
TBLK = 512
NT = T // TBLK                 # 4 token blocks

S_M = 16.0                     # fp8 scale on m1/m3 (a-values, sigma ~0.9)
S_B = 1024.0                   # fp8 scale on b1/b3 (weights, sigma ~0.02)
SCALE = S_M * S_B              # lora PSUM scale; folded into silu + rw


def build_nc(repeat=None, spec=None):
    """Build the per-core Bass module.

    spec: per-block tuple of active er-chunks for slot 0 (from the host's
    exact-expert token sort); None means dense (0, 1) everywhere.
    """
    nc = bacc.Bacc(None)

    # x is host-packed in h-chunk pairs [NH//2, 128, 2, T]; one DMA feeds
    # two SBUF chunk-views.
    xT = nc.declare_dram_parameter("xT", [NH // 2, 128, 2, T], BF16, isOutput=False)
    # w1/w3 are host-packed F-CHUNK-major: [NF, 128, NH, 128] — w1t[f][k,hc,m]
    # = W1[shard_f*128+m, hc*128+k]. One DMA per f-chunk means block-0's
    # phase A consumes weights in arrival order (A(f) needs only chunk f).
    w1t = nc.declare_dram_parameter("w1t", [NF, 128, NH, 128], BF16, isOutput=False)
    w3t = nc.declare_dram_parameter("w3t", [NF, 128, NH, 128], BF16, isOutput=False)
    w2t = nc.declare_dram_parameter("w2t", [NF, 128, H], BF16, isOutput=False)
    # fp8 LoRA operands: m{1,3} [K, 128, NER, T] (x S_M), b{1,3} [128, NER, FS]
    # (x S_B). Layout matches DoubleRow's [Ki, Ko=2, dim] AP: partition k is
    # er-row c*128+k of chunk c.
    m1t = nc.declare_dram_parameter("m1t", [K, 128, NER, T], F8E4, isOutput=False)
    m3t = nc.declare_dram_parameter("m3t", [K, 128, NER, T], F8E4, isOutput=False)
    b1t = nc.declare_dram_parameter("b1t", [128, NER, FS], F8E4, isOutput=False)
    b3t = nc.declare_dram_parameter("b3t", [128, NER, FS], F8E4, isOutput=False)
    a2t = nc.declare_dram_parameter("a2t", [NF, 128, ER], BF16, isOutput=False)
    rwr = nc.declare_dram_parameter("rwr", [K, 1, T], BF16, isOutput=False)
    outT = nc.declare_dram_parameter("outT", [NH, 128, T], BF16, isOutput=True)
    m2o = nc.declare_dram_parameter("m2o", [K, NER, 128, T], BF16, isOutput=True)

    with tile.TileContext(nc) as tc, ExitStack() as ctx:
        resw = ctx.enter_context(tc.tile_pool(name="resw", bufs=1))
        xsp = ctx.enter_context(tc.tile_pool(name="xsp", bufs=2))
        actp = ctx.enter_context(tc.tile_pool(name="actp", bufs=1))
        mp_ = ctx.enter_context(tc.tile_pool(name="mp", bufs=2))
        trans = ctx.enter_context(tc.tile_pool(name="trans", bufs=4))
        outp = ctx.enter_context(tc.tile_pool(name="outp", bufs=4))
        # PSUM partition: phase A gets 4 banks (2 tags x 2 bufs), phase B's
        # short LoRA groups 2 banks, a2 + down-proj share 2 banks.
        psp = ctx.enter_context(tc.tile_pool(name="psp", bufs=2, space="PSUM"))
        pspB = ctx.enter_context(tc.tile_pool(name="pspB", bufs=1, space="PSUM"))
        pspD = ctx.enter_context(tc.tile_pool(name="pspD", bufs=2, space="PSUM"))

        loop_cm = tc.For_i(0, repeat, 1) if repeat is not None else None
        if loop_cm is not None:
            loop_cm.__enter__()

        # ---- per-block input streamers ----
        def load_block_inputs(tb, xs=None):
            tsl = slice(tb * TBLK, (tb + 1) * TBLK)
            if xs is None:
                xs = []
                for hp in range(NH // 2):
                    xt_ = xsp.tile([128, 2 * TBLK], BF16, name=f"x{hp}",
                                   tag=f"x{hp}")
                    nc.sync.dma_start(out=xt_, in_=xT[hp][:, :, tsl])
                    xs.append(xt_[:, 0:TBLK])
                    xs.append(xt_[:, TBLK:2 * TBLK])
            m1, m3 = [None] * K, [None] * K
            for k in range(K):
                m1_ = mp_.tile([128, NER, TBLK], F8E4, name=f"m1_{k}",
                               tag=f"m1_{k}")
                nc.sync.dma_start(out=m1_, in_=m1t[k][:, :, tsl])
                m1[k] = m1_
                m3_ = mp_.tile([128, NER, TBLK], F8E4, name=f"m3_{k}",
                               tag=f"m3_{k}")
                nc.scalar.dma_start(out=m3_, in_=m3t[k][:, :, tsl])
                m3[k] = m3_
            rws = []
            for k in range(K):
                r_ = mp_.tile([128, TBLK], BF16, name=f"rw{k}", tag=f"rw{k}")
                nc.sync.dma_start(out=r_, in_=rwr[k][:, tsl].to_broadcast([128, TBLK]))
                rws.append(r_)
            return xs, rws, m1, m3

        # ---- resident weights, emitted in CONSUMPTION order so the two
        # ~170GB/s HWDGE rings deliver each tensor just before phase A of
        # block 0 needs it: f0 (split in h-halves for the earliest first
        # matmul) interleaved with the first x pairs, then f1..f6, then the
        # lora/m inputs, then w2/a2 (needed ~60us in). ----
        w1f, w3f = [None] * NF, [None] * NF
        NQ = NH // 4
        w1f0, w3f0 = [None] * 4, [None] * 4   # h-quarters of f-chunk 0

        def load_wf0_quarter(i):
            hs = slice(i * NQ, (i + 1) * NQ)
            t1 = resw.tile([128, NQ, 128], BF16, name=f"w1f0{i}",
                           tag=f"w1f0{i}")
            nc.sync.dma_start(out=t1, in_=w1t[0][:, hs, :])
            w1f0[i] = t1
            t3 = resw.tile([128, NQ, 128], BF16, name=f"w3f0{i}",
                           tag=f"w3f0{i}")
            nc.scalar.dma_start(out=t3, in_=w3t[0][:, hs, :])
            w3f0[i] = t3

        def load_wf(f):
            t1 = resw.tile([128, NH, 128], BF16, name=f"w1f{f}", tag=f"w1f{f}")
            nc.sync.dma_start(out=t1, in_=w1t[f])
            w1f[f] = t1
            t3 = resw.tile([128, NH, 128], BF16, name=f"w3f{f}", tag=f"w3f{f}")
            nc.scalar.dma_start(out=t3, in_=w3t[f])
            w3f[f] = t3

        def w1v(f, h):
            if f == 0:
                return w1f0[h // NQ][:, h % NQ, :]
            return w1f[f][:, h, :]

        def w3v(f, h):
            if f == 0:
                return w3f0[h // NQ][:, h % NQ, :]
            return w3f[f][:, h, :]

        xs0 = []

        def load_x0_pair(hp):
            xt_ = xsp.tile([128, 2 * TBLK], BF16, name=f"x{hp}", tag=f"x{hp}")
            xeng = nc.sync if hp % 2 == 0 else nc.scalar
            xeng.dma_start(out=xt_, in_=xT[hp][:, :, 0:TBLK])
            xs0.append(xt_[:, 0:TBLK])
            xs0.append(xt_[:, TBLK:2 * TBLK])

        load_wf0_quarter(0)
        load_x0_pair(0); load_x0_pair(1)
        load_wf0_quarter(1)
        load_x0_pair(2); load_x0_pair(3)
        load_wf0_quarter(2)
        load_x0_pair(4); load_x0_pair(5)
        load_wf0_quarter(3)
        load_x0_pair(6); load_x0_pair(7)
        for f in range(1, NF):
            load_wf(f)
        b1s = resw.tile([128, NER, FS], F8E4, name="b1s", tag="b1s")
        nc.sync.dma_start(out=b1s, in_=b1t[:, :, :])
        b3s = resw.tile([128, NER, FS], F8E4, name="b3s", tag="b3s")
        nc.scalar.dma_start(out=b3s, in_=b3t[:, :, :])
        pre0 = load_block_inputs(0, xs0)
        w2s, a2s = [], []
        for f in range(NF):
            eng = nc.sync if f % 2 == 0 else nc.scalar
            t_ = resw.tile([128, H], BF16, name=f"w2s{f}", tag=f"w2s{f}")
            eng.dma_start(out=t_, in_=w2t[f])
            w2s.append(t_)
        for f in range(NF):
            eng = nc.scalar if f % 2 == 0 else nc.sync
            t_ = resw.tile([128, ER], BF16, name=f"a2s{f}", tag=f"a2s{f}")
            eng.dma_start(out=t_, in_=a2t[f])
            a2s.append(t_)

        # ---- phase emitters (software-pipelined across blocks below) ----
        def emit_A_group(xs, f, base1, base3):
            """One f-chunk of base1/base3 = W1/W3 @ x (PE-dense, no deps).
            The PSUM->SBUF copies scale by SCALE so phase B's adds work in
            the fp8-product scale with zero extra ops. The last f-chunk
            rides the pD banks (idle right after a2) so the block's first
            C-group leader on pA waits one A-copy less in the DVE queue."""
            if f == NF - 1:
                ps1 = pspD.tile([128, TBLK], F32, name="ps1", tag="pD")
                ps3 = pspD.tile([128, TBLK], F32, name="ps3", tag="pD")
            else:
                ps1 = psp.tile([128, TBLK], F32, name="ps1", tag="pA")
                ps3 = psp.tile([128, TBLK], F32, name="ps3", tag="pB")
            for h in range(NH):
                nc.tensor.matmul(ps1, w1v(f, h), xs[h], start=(h == 0), stop=(h == NH - 1))
                nc.tensor.matmul(ps3, w3v(f, h), xs[h], start=(h == 0), stop=(h == NH - 1))
            # copies ride the DVE: the scalar engine's stream is clogged by
            # DMA-issue instructions early on (queue-full waits), and a
            # scalar copy here would delay the PSUM bank release that gates
            # the next A group's leader matmul.
            b1_ = actp.tile([128, TBLK], BF16, name=f"b1_{f}", tag=f"b1_{f}")
            nc.vector.tensor_scalar_mul(b1_, ps1, SCALE)
            base1[f] = b1_
            b3_ = actp.tile([128, TBLK], BF16, name=f"b3_{f}", tag=f"b3_{f}")
            nc.vector.tensor_scalar_mul(b3_, ps3, SCALE)
            base3[f] = b3_

        def emit_A(xs):
            base1, base3 = [None] * NF, [None] * NF
            for f in range(NF):
                emit_A_group(xs, f, base1, base3)
            return base1, base3

        def emit_xsum(f, x2s, xsum):
            xs_ = actp.tile([128, TBLK], BF16, name=f"xsum{f}",
                            tag=f"xsum{f}")
            # alternate gpsimd/DVE: gpsimd's ~2.3us per add serializes all
            # seven xsums, and xsum[6] gates the block's first C group
            eng = nc.gpsimd if f % 2 == 0 else nc.vector
            eng.tensor_add(xs_, x2s[0][f], x2s[1][f])
            xsum[f] = xs_

        def segs_of(tb, k):
            """Column segments (er_chunk, c0, c1) covering the block. The
            host sub-sorts tokens inside each block by (slot0-chunk,
            slot1-chunk), so every token column is contracted against ONLY
            its own er-chunk — the LoRA matmul work per column halves vs
            contracting both chunks everywhere."""
            if spec is None:
                return ((0, 0, TBLK), (1, 0, TBLK))
            return spec[tb][k]

        def emit_B_f(tb, k, f, base1, base3, rws, m1, m3, x2s, xsum,
                     do_xsum=True, altps=False):
            """LoRA up-proj + silu/mul chain for one (slot, f-chunk).
            altps (last block): alternate the PSUM tags between pspB's
            qA/qB and psp's idle pA/pB so each tag is reused every OTHER
            iteration — the DVE chain then never gates the leader matmul."""
            segs = segs_of(tb, k)
            fsl = slice(f * 128, (f + 1) * 128)
            if altps and f % 2 == 1:
                psA = psp.tile([128, TBLK], F32, name="psA", tag="pA")
                psB = psp.tile([128, TBLK], F32, name="psB", tag="pB")
            else:
                psA = pspB.tile([128, TBLK], F32, name="psA", tag="qA")
                psB = pspB.tile([128, TBLK], F32, name="psB", tag="qB")
            for (er, c0, c1) in segs:
                nc.tensor.matmul(psA[:, c0:c1], b1s[:, er, fsl],
                                 m1[k][:, er, c0:c1], start=True, stop=True)
            for (er, c0, c1) in segs:
                nc.tensor.matmul(psB[:, c0:c1], b3s[:, er, fsl],
                                 m3[k][:, er, c0:c1], start=True, stop=True)
            t1_ = trans.tile([128, TBLK], BF16, name="t1", tag="t1")
            nc.vector.tensor_add(t1_, psA, base1[f])
            sl_ = trans.tile([128, TBLK], BF16, name="sl", tag="sl")
            nc.scalar.activation(sl_, t1_, mybir.ActivationFunctionType.Silu,
                                 scale=1.0 / SCALE)
            t3_ = trans.tile([128, TBLK], BF16, name="t3", tag="t3")
            nc.vector.tensor_add(t3_, psB, base3[f])
            x3s_ = trans.tile([128, TBLK], BF16, name="x3s", tag="x3s")
            nc.vector.tensor_mul(x3s_, t3_, rws[k])
            x2_ = actp.tile([128, TBLK], BF16, name=f"x2_{k}{f}",
                            tag=f"x2_{k}{f}")
            nc.vector.tensor_mul(x2_, sl_, x3s_)
            x2s[k][f] = x2_
            if k == K - 1 and do_xsum:
                emit_xsum(f, x2s, xsum)

        def emit_a2(tb, k, x2s):
            t0 = tb * TBLK
            segs = segs_of(tb, k)
            for er in range(NER):
                ranges = [(c0, c1) for (e, c0, c1) in segs if e == er]
                if not ranges:
                    continue
                ers = slice(er * 128, (er + 1) * 128)
                psa2 = pspD.tile([128, TBLK], F32, name="psa2", tag="pD")
                for (c0, c1) in ranges:
                    for f in range(NF):
                        nc.tensor.matmul(psa2[:, c0:c1], a2s[f][:, ers],
                                         x2s[k][f][:, c0:c1],
                                         start=(f == 0), stop=(f == NF - 1))
                m2_ = actp.tile([128, TBLK], BF16, name=f"m2_{k}{er}",
                                tag=f"m2_{k}{er}")
                eng = nc.scalar if (k + er) % 2 == 0 else nc.sync
                # copy/DMA only the covered ranges: the uncovered psa2
                # columns are stale PSUM (m2o is zero-initialized and the
                # host masks per-token, so untouched regions contribute 0).
                for (c0, c1) in ranges:
                    nc.scalar.copy(m2_[:, c0:c1], psa2[:, c0:c1])
                    eng.dma_start(out=m2o[k][er][:, t0 + c0:t0 + c1],
                                  in_=m2_[:, c0:c1])

        def emit_C_group(tb, xsum, h, lastblk=False):
            tsl = slice(tb * TBLK, (tb + 1) * TBLK)
            hsl = slice(h * 128, (h + 1) * 128)
            if lastblk:
                # pA/pB belong to the last block's Bx iterations there
                psD = pspD.tile([128, TBLK], F32, name="psD", tag="pD")
            else:
                # ride the phase-A banks (idle once A'(5)/A'(6) have been
                # copied out): double-buffered leaders instead of sharing
                # pD's 2 banks with a2
                psD = psp.tile([128, TBLK], F32, name="psD",
                               tag=("pA" if h % 2 == 0 else "pB"))
            for f in range(NF):
                nc.tensor.matmul(psD, w2s[f][:, hsl], xsum[f],
                                 start=(f == 0), stop=(f == NF - 1))
            o_ = outp.tile([128, TBLK], BF16, name="osb", tag="osb")
            nc.scalar.copy(o_, psD)
            eng = nc.sync if h % 2 == 0 else nc.scalar
            eng.dma_start(out=outT[h][:, tsl], in_=o_)

        def emit_C_group_split(tb, xsum, h):
            """Very last C group, column-split with TWO half-bank PSUM
            tiles so half-1's copy+DMA overlap half-2's matmuls (a single
            tile serializes on Tile's per-tile read/write tracking)."""
            t0 = tb * TBLK
            hsl = slice(h * 128, (h + 1) * 128)
            HB = TBLK // 2
            for i, c0 in enumerate((0, HB)):
                psD = pspD.tile([128, HB], F32, name=f"psDs{i}", tag="pD")
                for f in range(NF):
                    nc.tensor.matmul(psD, w2s[f][:, hsl],
                                     xsum[f][:, c0:c0 + HB],
                                     start=(f == 0), stop=(f == NF - 1))
                o_ = outp.tile([128, HB], BF16, name="osbs", tag="osb")
                nc.scalar.copy(o_, psD)
                eng = nc.sync if i == 0 else nc.scalar
                eng.dma_start(out=outT[h][:, t0 + c0:t0 + c0 + HB], in_=o_)

        def emit_C(tb, xsum, last=False):
            for h in range(NH):
                if last and h == NH - 1:
                    emit_C_group_split(tb, xsum, h)
                else:
                    emit_C_group(tb, xsum, h, lastblk=last)

        # ---- software pipeline: next block's phase-A groups are woven
        # BETWEEN this block's phase-B iterations (PE executes in emission
        # order, so independent work must be emitted before gated work).
        # Phase B is Vector-throughput-bound (~2.2us of DVE chain per
        # f-iteration); each interposed A group gives the DVE ~7us of
        # matmul cover to drain its chain backlog, so the B-group PSUM
        # leaders never wait on bank release. ----
        xs, rws, m1, m3 = pre0
        base1, base3 = emit_A(xs)
        heldC = None   # xsum of block NT-2, its C woven into the last block
        for tb in range(NT):
            x2s = [[None] * NF for _ in range(K)]
            xsum = [None] * NF
            Bf = lambda k, f: emit_B_f(tb, k, f, base1, base3, rws, m1, m3,
                                       x2s, xsum)
            if tb + 1 < NT:
                xsn, rwsn, m1n, m3n = load_block_inputs(tb + 1)
                b1n, b3n = [None] * NF, [None] * NF
                A = lambda f: emit_A_group(xsn, f, b1n, b3n)
                Bf(0, 0); Bf(0, 1)
                Bf(0, 2); Bf(0, 3); A(0)
                Bf(0, 4); Bf(0, 5); A(1)
                Bf(0, 6); Bf(1, 0); A(2)
                Bf(1, 1); Bf(1, 2); A(3)
                Bf(1, 3); Bf(1, 4); emit_a2(tb, 0, x2s)
                Bf(1, 5); Bf(1, 6); A(4)
                emit_a2(tb, 1, x2s)
                A(5); A(6)
                xs, rws, m1, m3 = xsn, rwsn, m1n, m3n
                base1, base3 = b1n, b3n
                if tb == NT - 2:
                    heldC = xsum      # defer C(NT-2) into the last block
                else:
                    emit_C(tb, xsum)
            else:
                # last block has no next-A cover; weave the held-back
                # C(NT-2) groups among the k=0 iterations instead. All held
                # groups must be emitted before B(1,0) writes xsum (the
                # single-buffered xsum tags roll over to this block there).
                hq = list(range(NH))
                C2 = lambda n: [emit_C_group(tb - 1, heldC, hq.pop(0),
                                             lastblk=True)
                                for _ in range(n)]
                Bx = lambda k, f: emit_B_f(tb, k, f, base1, base3, rws, m1,
                                           m3, x2s, xsum, do_xsum=False,
                                           altps=True)
                Bx(0, 0); Bx(1, 0); C2(3)
                Bx(0, 1); Bx(1, 1); C2(3)
                Bx(0, 2); Bx(1, 2); C2(2)
                Bx(0, 3); Bx(1, 3); C2(2)
                Bx(0, 4); Bx(1, 4); C2(2)
                Bx(0, 5); Bx(1, 5); C2(2)
                Bx(0, 6); Bx(1, 6); C2(2)
                for f in range(NF):
                    emit_xsum(f, x2s, xsum)
                emit_a2(tb, 0, x2s)
                emit_a2(tb, 1, x2s)
                emit_C(tb, xsum, last=True)

        if loop_cm is not None:
            loop_cm.__exit__(None, None, None)

    nc.finalize()
    return nc


def _q8(a, scale):
    return np.clip(a * scale, -240.0, 240.0).astype(NPF8E4)


def prepare_inputs(hidden_states, Wg, W1, W2, W3, A1, B1, A2, B2, A3, B3):
    """Host preprocessing: routing + per-core weight slicing/casting."""
    hidden_states, Wg, W1, W2, W3, A1, B1, A2, B2, A3, B3 = (
        np.asarray(a, dtype=np.float32)
        for a in (hidden_states, Wg, W1, W2, W3, A1, B1, A2, B2, A3, B3))
    x = np.ascontiguousarray(hidden_states.reshape(T, H))

    logits = x @ Wg.T.astype(np.float32)
    m = logits.max(-1, keepdims=True)
    p = np.exp(logits - m, dtype=np.float32)
    p /= p.sum(-1, keepdims=True)
    sel = np.argsort(-p, axis=-1, kind="stable")[:, :K]      # [T, K]
    rw = np.take_along_axis(p, sel, axis=1)
    rw = (rw / rw.sum(-1, keepdims=True)).astype(np.float32)  # [T, K]

    # Sort tokens by slot-0 EXPERT (block composition), then inside each
    # block by (slot0-chunk, slot1-chunk). Each block then decomposes into
    # a few contiguous column segments per slot, each touching a single
    # er-chunk — the device contracts every token column against only ITS
    # chunk instead of both.
    GE = E // NER                         # experts per er-chunk
    perm = np.argsort(sel[:, 0], kind="stable")
    for b in range(NT):
        idx = perm[b * TBLK:(b + 1) * TBLK]
        key = (sel[idx, 0] // GE) * NER + (sel[idx, 1] // GE)
        perm[b * TBLK:(b + 1) * TBLK] = idx[np.argsort(key, kind="stable")]
    x = np.ascontiguousarray(x[perm])
    sel = sel[perm]
    rw = np.ascontiguousarray(rw[perm])

    spec = []
    for b in range(NT):
        per_slot = []
        for k in range(K):
            ch = sel[b * TBLK:(b + 1) * TBLK, k] // GE
            segs, start = [], 0
            for i in range(1, TBLK + 1):
                if i == TBLK or ch[i] != ch[i - 1]:
                    segs.append((int(ch[start]), start, i))
                    start = i
            per_slot.append(tuple(segs))
        spec.append(tuple(per_slot))
    spec = tuple(spec)

    xT_np = np.ascontiguousarray(
        x.T.reshape(NH // 2, 2, 128, T).transpose(0, 2, 1, 3)
    ).astype(NPBF16)                                  # [NH//2, 128, 2, T]

    # per-slot one-hot masks over the (e, r) axis, transposed to [ER, T];
    # applied HOST-side to the returned a2 (masking is elementwise, so it
    # commutes with the cross-core partial sum)
    masks = np.zeros((K, ER, T), dtype=np.float32)
    for k in range(K):
        onehot = np.zeros((T, E), np.float32)
        onehot[np.arange(T), sel[:, k]] = 1.0
        masks[k] = np.repeat(onehot, R, axis=1).T
    # rw is pre-divided by SCALE: the device's x3 path multiplies the
    # SCALE-scaled (base3 + lora3) PSUM values by it, landing on true scale.
    rwr_np = np.ascontiguousarray(rw.T / SCALE).reshape(K, 1, T).astype(NPBF16)

    # flattened LoRA tensors (full copies; small)
    A1f = A1.reshape(ER, H)                      # [er, H]
    A3f = A3.reshape(ER, H)
    B2f = B2.transpose(0, 2, 1).reshape(ER, H)   # [er, H]

    # per-slot masked LoRA down-projections, computed host-side in fp32,
    # quantized to fp8e4 (x S_M) in the DoubleRow [Ki=128, Ko=NER, T] layout
    a1_all = x @ A1f.T.astype(np.float32)        # [T, ER]
    a3_all = x @ A3f.T.astype(np.float32)
    m1t_np = np.zeros((K, ER, T), dtype=NPF8E4)
    m3t_np = np.zeros((K, ER, T), dtype=NPF8E4)
    for k in range(K):
        mx = np.repeat(
            np.eye(E, dtype=np.float32)[sel[:, k]], R, axis=1)   # [T, ER]
        m1t_np[k] = _q8((a1_all * mx).T, S_M)
        m3t_np[k] = _q8((a3_all * mx).T, S_M)
    m1t_np = np.ascontiguousarray(
        m1t_np.reshape(K, NER, 128, T).transpose(0, 2, 1, 3))
    m3t_np = np.ascontiguousarray(
        m3t_np.reshape(K, NER, 128, T).transpose(0, 2, 1, 3))

    def pack_fmajor(wT):
        # [FS, H] -> [NF, 128, NH, 128]: [f, k, hc, m] = W[f*128+m, hc*128+k]
        return np.ascontiguousarray(
            wT.reshape(NF, 128, NH, 128).transpose(0, 3, 2, 1))

    in_maps = []
    for c in range(NCORES):
        fs = slice(c * FS, (c + 1) * FS)
        w1t_np = pack_fmajor(W1[fs]).astype(NPBF16)
        w3t_np = pack_fmajor(W3[fs]).astype(NPBF16)
        w2T = np.ascontiguousarray(W2[:, fs].T).astype(NPBF16)  # [FS, H]
        w2t_np = w2T.reshape(NF, 128, H)
        b1f = B1[:, fs, :].transpose(0, 2, 1).reshape(ER, FS)   # [er, f]
        b3f = B3[:, fs, :].transpose(0, 2, 1).reshape(ER, FS)
        b1t_np = np.ascontiguousarray(
            _q8(b1f, S_B).reshape(NER, 128, FS).transpose(1, 0, 2))
        b3t_np = np.ascontiguousarray(
            _q8(b3f, S_B).reshape(NER, 128, FS).transpose(1, 0, 2))
        a2f = A2[:, :, fs].reshape(ER, FS)                      # [er, f]
        a2t_np = np.ascontiguousarray(a2f.T).astype(NPBF16).reshape(NF, 128, ER)

        in_maps.append({
            "xT": xT_np, "w1t": w1t_np, "w3t": w3t_np, "w2t": w2t_np,
            "m1t": m1t_np, "m3t": m3t_np, "b1t": b1t_np, "b3t": b3t_np,
            "a2t": a2t_np,
            "rwr": rwr_np,
        })
    return in_maps, (B2f.astype(np.float32), masks, perm, spec)


_CACHED_NC = {}


def kernel(hidden_states, Wg, W1, W2, W3, A1, B1, A2, B2, A3, B3,
           _trace=False, _tmpdir=None):
    in_maps, (B2f, masks, perm, spec) = prepare_inputs(
        hidden_states, Wg, W1, W2, W3, A1, B1, A2, B2, A3, B3)
    if spec not in _CACHED_NC:
        _CACHED_NC[spec] = build_nc(spec=spec)
    nc = _CACHED_NC[spec]
    res = run_bass_kernel_spmd(nc, in_maps, list(range(NCORES)),
                               trace=_trace, tmpdir=_tmpdir)
    acc = np.zeros((NH, 128, T), np.float32)
    m2sum = np.zeros((K, ER, T), np.float32)
    for c in range(NCORES):
        acc += res.results[c]["outT"].astype(np.float32)
        m2sum += res.results[c]["m2o"].reshape(K, ER, T).astype(np.float32)
    out = acc.reshape(H, T)
    # host-side lora2: mask the (unmasked, core-summed) a2, then the final
    # LoRA up-projection is linear -> one small GEMM per slot
    for k in range(K):
        out += B2f.T @ (m2sum[k] * masks[k])
    outT_tok = out.T                       # [T, H], token-permuted order
    final = np.empty_like(outT_tok)
    final[perm] = outT_tok                 # undo the expert sort
    out = final.reshape(B, S, H)
    kernel.last_results = res
    return out


if __name__ == "__main__":
    nc = build_nc(spec=None)
    print("built ok")


# revision 38
# speedup vs baseline: 1.0079x; 1.0059x over previous
"""Mixtral sparse-MoE block with per-expert LoRA adapters on 8 Trainium2 cores.

Problem shapes: B=2, S=1024, H=2048, F=7168, E=8, R=32, top-K=2.
T = B*S = 2048 tokens.

Sharding: tensor-parallel over the FFN dim F. Core c owns rows
[c*896:(c+1)*896] of W1/W3 (and the matching B1/B3 LoRA rows) and the same
columns of W2/A2. Everything after the silu is linear in
x2s = silu(x1)*x3*rw, so each core emits an exact partial [H, T] output over
its F-shard (bf16) and the host sums the 8 partials in fp32.

Work split (device vs host):
- Host: gating (softmax + top-2), the tiny per-expert LoRA down-projections
  a1/a3 = x @ A{1,3}T masked per slot (m1/m3 uploads, fp8e4), and the final
  LoRA up-projection lora2 = B2 @ sum_cores(m2) (one small GEMM).
- Device (per core): base1/base3 = x @ W{1,3}[shard].T, the per-slot LoRA
  up-projections lora1/3 = B{1,3}[shard] @ m{1,3} accumulated in PSUM,
  silu/mul/scale chain, a2 = A2[shard] @ x2s per slot (m2, returned to
  host), and the down-projection on the slot-summed activations.

Perf structure (measured on trn2 via NTFF hardware traces):
- PE-bound: a dense bf16 MM stream runs ~216ns per K=128/N=512 matmul at
  2.4GHz. Reductions vs the v1 kernel:
  * w1/w3 are streamed F-CHUNK-major ([NF,128,NH,128] host packs): block-0
    phase A consumes weights in DMA arrival order, so the PE starts ~5us
    earlier and the ~7us of warmup stalls (waiting for h-major pair DMAs)
    disappear.
  * Tokens are sorted by slot-0 expert, then inside each block by
    (slot0-chunk, slot1-chunk). Each block decomposes into contiguous
    column segments per slot that touch a single er-chunk, so the LoRA
    up-projections and a2 contract every token column against only ITS
    er-chunk: ~half the LoRA matmul columns vs dense. (fp8 DoubleRow was
    tried for the dense case and measured at only ~6% over two bf16
    matmuls — column splitting beats it and needs no perf mode.)
  * m1/m3/b1/b3 are host-quantized fp8e4 with power-of-2 scales (S_M=16,
    S_B=1024); the product scale 1/16384 is folded into the silu's input
    scale and the host-prescaled rw tensor — zero dequant ops on device.
  * outT is written bf16 (halves the output drain) and output DMAs
    alternate between the sync/scalar HWDGE rings.
  * Phase-A PSUM->SBUF copies run on the DVE, not the scalar engine: the
    scalar engine's stream is clogged by DMA-issue instructions during
    warmup (each blocks on HWDGE queue space), which held PSUM banks
    hostage and stalled phase A for ~17us.
- The block loop is software-pipelined: block b+1's phase-A groups are
  EMITTED between block b's phase-B iterations. Phase B is DVE-bound
  (~2.2us of add/silu/mul chain per f-iteration), and the PE executes in
  emission order, so each interposed phase-A group gives the DVE a ~7us
  matmul window to drain its backlog. The last block (no next A) instead
  weaves in the held-back phase-C groups of block NT-2.
- Every dma_start costs ~0.6us of ISSUE time on its HWDGE ring regardless
  of size; streamed tensors are host-packed so one DMA feeds multiple SBUF
  chunk views. Only sync/scalar HWDGE rings are used.
- Outputs are unmasked a2 partials; the (elementwise) expert mask commutes
  with the cross-core sum and is applied on host before the lora2 GEMM.
"""

import sys
from contextlib import ExitStack

import numpy as np

try:
    import concourse.bass as bass  # noqa: F401
except ImportError:
    sys.path.insert(0, "/opt/trn_rl_repo")

import ml_dtypes

import concourse.bass as bass
import concourse.mybir as mybir
import concourse.tile as tile
from concourse import bacc
from concourse.bass_utils import run_bass_kernel_spmd

BF16 = mybir.dt.bfloat16
F8E4 = mybir.dt.float8e4
F32 = mybir.dt.float32
NPBF16 = ml_dtypes.bfloat16
NPF8E4 = ml_dtypes.float8_e4m3

B, S, H, F, E, R, K = 2, 1024, 2048, 7168, 8, 32, 2
T = B * S                      # 2048 tokens
ER = E * R                     # 256
NCORES = 8
FS = F // NCORES               # 896 per-core F shard
NH = H // 128                  # 16 h-chunks
NF = FS // 128                 # 7 f-chunks (per core)
NER = ER // 128                # 2 er-chunks
TBLK = 512
NT = T // TBLK                 # 4 token blocks

S_M = 16.0                     # fp8 scale on m1/m3 (a-values, sigma ~0.9)
S_B = 1024.0                   # fp8 scale on b1/b3 (weights, sigma ~0.02)
SCALE = S_M * S_B              # lora PSUM scale; folded into silu + rw


def build_nc(repeat=None, spec=None):
    """Build the per-core Bass module.

    spec: per-block tuple of active er-chunks for slot 0 (from the host's
    exact-expert token sort); None means dense (0, 1) everywhere.
    """
    nc = bacc.Bacc(None)

    # x is host-packed in h-chunk pairs [NH//2, 128, 2, T]; one DMA feeds
    # two SBUF chunk-views.
    xT = nc.declare_dram_parameter("xT", [NH // 2, 128, 2, T], BF16, isOutput=False)
    # w1/w3 are host-packed F-CHUNK-major: [NF, 128, NH, 128] — w1t[f][k,hc,m]
    # = W1[shard_f*128+m, hc*128+k]. One DMA per f-chunk means block-0's
    # phase A consumes weights in arrival order (A(f) needs only chunk f).
    w1t = nc.declare_dram_parameter("w1t", [NF, 128, NH, 128], BF16, isOutput=False)
    w3t = nc.declare_dram_parameter("w3t", [NF, 128, NH, 128], BF16, isOutput=False)
    w2t = nc.declare_dram_parameter("w2t", [NF, 128, H], BF16, isOutput=False)
    # fp8 LoRA operands: m{1,3} [K, 128, NER, T] (x S_M), b{1,3} [128, NER, FS]
    # (x S_B). Layout matches DoubleRow's [Ki, Ko=2, dim] AP: partition k is
    # er-row c*128+k of chunk c.
    m1t = nc.declare_dram_parameter("m1t", [K, 128, NER, T], F8E4, isOutput=False)
    m3t = nc.declare_dram_parameter("m3t", [K, 128, NER, T], F8E4, isOutput=False)
    b1t = nc.declare_dram_parameter("b1t", [128, NER, FS], F8E4, isOutput=False)
    b3t = nc.declare_dram_parameter("b3t", [128, NER, FS], F8E4, isOutput=False)
    a2t = nc.declare_dram_parameter("a2t", [NF, 128, ER], BF16, isOutput=False)
    rwr = nc.declare_dram_parameter("rwr", [K, 1, T], BF16, isOutput=False)
    outT = nc.declare_dram_parameter("outT", [NH, 128, T], BF16, isOutput=True)
    m2o = nc.declare_dram_parameter("m2o", [K, NER, 128, T], BF16, isOutput=True)

    with tile.TileContext(nc) as tc, ExitStack() as ctx:
        resw = ctx.enter_context(tc.tile_pool(name="resw", bufs=1))
        xsp = ctx.enter_context(tc.tile_pool(name="xsp", bufs=2))
        actp = ctx.enter_context(tc.tile_pool(name="actp", bufs=1))
        mp_ = ctx.enter_context(tc.tile_pool(name="mp", bufs=2))
        trans = ctx.enter_context(tc.tile_pool(name="trans", bufs=4))
        outp = ctx.enter_context(tc.tile_pool(name="outp", bufs=4))
        # PSUM partition: phase A gets 4 banks (2 tags x 2 bufs), phase B's
        # short LoRA groups 2 banks, a2 + down-proj share 2 banks.
        psp = ctx.enter_context(tc.tile_pool(name="psp", bufs=2, space="PSUM"))
        pspB = ctx.enter_context(tc.tile_pool(name="pspB", bufs=1, space="PSUM"))
        pspD = ctx.enter_context(tc.tile_pool(name="pspD", bufs=2, space="PSUM"))

        loop_cm = tc.For_i(0, repeat, 1) if repeat is not None else None
        if loop_cm is not None:
            loop_cm.__enter__()

        # ---- per-block input streamers ----
        def load_block_inputs(tb, xs=None):
            tsl = slice(tb * TBLK, (tb + 1) * TBLK)
            if xs is None:
                xs = []
                for hp in range(NH // 2):
                    xt_ = xsp.tile([128, 2 * TBLK], BF16, name=f"x{hp}",
                                   tag=f"x{hp}")
                    nc.sync.dma_start(out=xt_, in_=xT[hp][:, :, tsl])
                    xs.append(xt_[:, 0:TBLK])
                    xs.append(xt_[:, TBLK:2 * TBLK])
            m1, m3 = [None] * K, [None] * K
            for k in range(K):
                m1_ = mp_.tile([128, NER, TBLK], F8E4, name=f"m1_{k}",
                               tag=f"m1_{k}")
                nc.sync.dma_start(out=m1_, in_=m1t[k][:, :, tsl])
                m1[k] = m1_
                m3_ = mp_.tile([128, NER, TBLK], F8E4, name=f"m3_{k}",
                               tag=f"m3_{k}")
                nc.scalar.dma_start(out=m3_, in_=m3t[k][:, :, tsl])
                m3[k] = m3_
            rws = []
            for k in range(K):
                r_ = mp_.tile([128, TBLK], BF16, name=f"rw{k}", tag=f"rw{k}")
                nc.sync.dma_start(out=r_, in_=rwr[k][:, tsl].to_broadcast([128, TBLK]))
                rws.append(r_)
            return xs, rws, m1, m3

        # ---- resident weights, emitted in CONSUMPTION order so the two
        # ~170GB/s HWDGE rings deliver each tensor just before phase A of
        # block 0 needs it: f0 (split in h-halves for the earliest first
        # matmul) interleaved with the first x pairs, then f1..f6, then the
        # lora/m inputs, then w2/a2 (needed ~60us in). ----
        w1f, w3f = [None] * NF, [None] * NF
        NQ = NH // 4
        w1f0, w3f0 = [None] * 4, [None] * 4   # h-quarters of f-chunk 0

        def load_wf0_quarter(i):
            hs = slice(i * NQ, (i + 1) * NQ)
            t1 = resw.tile([128, NQ, 128], BF16, name=f"w1f0{i}",
                           tag=f"w1f0{i}")
            nc.sync.dma_start(out=t1, in_=w1t[0][:, hs, :])
            w1f0[i] = t1
            t3 = resw.tile([128, NQ, 128], BF16, name=f"w3f0{i}",
                           tag=f"w3f0{i}")
            nc.scalar.dma_start(out=t3, in_=w3t[0][:, hs, :])
            w3f0[i] = t3

        def load_wf(f):
            t1 = resw.tile([128, NH, 128], BF16, name=f"w1f{f}", tag=f"w1f{f}")
            nc.sync.dma_start(out=t1, in_=w1t[f])
            w1f[f] = t1
            t3 = resw.tile([128, NH, 128], BF16, name=f"w3f{f}", tag=f"w3f{f}")
            nc.scalar.dma_start(out=t3, in_=w3t[f])
            w3f[f] = t3

        # f-chunk 1 splits each tensor's h-halves ACROSS the rings: the
        # first halves land ~1.5us earlier than a single-ring full chunk,
        # which is exactly the margin A(1)'s leader was stalling on.
        w1f1h, w3f1h = [None] * 2, [None] * 2

        def load_wf1_split():
            ha = slice(0, NH // 2)
            hb = slice(NH // 2, NH)
            t1a = resw.tile([128, NH // 2, 128], BF16, name="w1f1a", tag="w1f1a")
            nc.sync.dma_start(out=t1a, in_=w1t[1][:, ha, :])
            w1f1h[0] = t1a
            t3a = resw.tile([128, NH // 2, 128], BF16, name="w3f1a", tag="w3f1a")
            nc.scalar.dma_start(out=t3a, in_=w3t[1][:, ha, :])
            w3f1h[0] = t3a
            t1b = resw.tile([128, NH // 2, 128], BF16, name="w1f1b", tag="w1f1b")
            nc.scalar.dma_start(out=t1b, in_=w1t[1][:, hb, :])
            w1f1h[1] = t1b
            t3b = resw.tile([128, NH // 2, 128], BF16, name="w3f1b", tag="w3f1b")
            nc.sync.dma_start(out=t3b, in_=w3t[1][:, hb, :])
            w3f1h[1] = t3b

        def w1v(f, h):
            if f == 0:
                return w1f0[h // NQ][:, h % NQ, :]
            if f == 1:
                return w1f1h[h // (NH // 2)][:, h % (NH // 2), :]
            return w1f[f][:, h, :]

        def w3v(f, h):
            if f == 0:
                return w3f0[h // NQ][:, h % NQ, :]
            if f == 1:
                return w3f1h[h // (NH // 2)][:, h % (NH // 2), :]
            return w3f[f][:, h, :]

        xs0 = []

        def load_x0_pair(hp):
            xt_ = xsp.tile([128, 2 * TBLK], BF16, name=f"x{hp}", tag=f"x{hp}")
            xeng = nc.sync if hp % 2 == 0 else nc.scalar
            xeng.dma_start(out=xt_, in_=xT[hp][:, :, 0:TBLK])
            xs0.append(xt_[:, 0:TBLK])
            xs0.append(xt_[:, TBLK:2 * TBLK])

        load_wf0_quarter(0)
        load_x0_pair(0); load_x0_pair(1)
        load_wf0_quarter(1)
        load_x0_pair(2); load_x0_pair(3)
        load_wf0_quarter(2)
        load_x0_pair(4); load_x0_pair(5)
        load_wf0_quarter(3)
        load_x0_pair(6); load_x0_pair(7)
        load_wf1_split()
        for f in range(2, NF):
            load_wf(f)
        b1s = resw.tile([128, NER, FS], F8E4, name="b1s", tag="b1s")
        nc.sync.dma_start(out=b1s, in_=b1t[:, :, :])
        b3s = resw.tile([128, NER, FS], F8E4, name="b3s", tag="b3s")
        nc.scalar.dma_start(out=b3s, in_=b3t[:, :, :])
        pre0 = load_block_inputs(0, xs0)
        w2s, a2s = [], []
        for f in range(NF):
            eng = nc.sync if f % 2 == 0 else nc.scalar
            t_ = resw.tile([128, H], BF16, name=f"w2s{f}", tag=f"w2s{f}")
            eng.dma_start(out=t_, in_=w2t[f])
            w2s.append(t_)
        for f in range(NF):
            eng = nc.scalar if f % 2 == 0 else nc.sync
            t_ = resw.tile([128, ER], BF16, name=f"a2s{f}", tag=f"a2s{f}")
            eng.dma_start(out=t_, in_=a2t[f])
            a2s.append(t_)

        # ---- phase emitters (software-pipelined across blocks below) ----
        def emit_A_group(xs, f, base1, base3):
            """One f-chunk of base1/base3 = W1/W3 @ x (PE-dense, no deps).
            The PSUM->SBUF copies scale by SCALE so phase B's adds work in
            the fp8-product scale with zero extra ops. The last f-chunk
            rides the pD banks (idle right after a2) so the block's first
            C-group leader on pA waits one A-copy less in the DVE queue."""
            if f == NF - 1:
                ps1 = pspD.tile([128, TBLK], F32, name="ps1", tag="pD")
                ps3 = pspD.tile([128, TBLK], F32, name="ps3", tag="pD")
            else:
                ps1 = psp.tile([128, TBLK], F32, name="ps1", tag="pA")
                ps3 = psp.tile([128, TBLK], F32, name="ps3", tag="pB")
            for h in range(NH):
                nc.tensor.matmul(ps1, w1v(f, h), xs[h], start=(h == 0), stop=(h == NH - 1))
                nc.tensor.matmul(ps3, w3v(f, h), xs[h], start=(h == 0), stop=(h == NH - 1))
            # copies ride the DVE: the scalar engine's stream is clogged by
            # DMA-issue instructions early on (queue-full waits), and a
            # scalar copy here would delay the PSUM bank release that gates
            # the next A group's leader matmul.
            b1_ = actp.tile([128, TBLK], BF16, name=f"b1_{f}", tag=f"b1_{f}")
            nc.vector.tensor_scalar_mul(b1_, ps1, SCALE)
            base1[f] = b1_
            b3_ = actp.tile([128, TBLK], BF16, name=f"b3_{f}", tag=f"b3_{f}")
            nc.vector.tensor_scalar_mul(b3_, ps3, SCALE)
            base3[f] = b3_

        def emit_A(xs):
            base1, base3 = [None] * NF, [None] * NF
            for f in range(NF):
                emit_A_group(xs, f, base1, base3)
            return base1, base3

        def emit_xsum(f, x2s, xsum):
            xs_ = actp.tile([128, TBLK], BF16, name=f"xsum{f}",
                            tag=f"xsum{f}")
            # alternate gpsimd/DVE: gpsimd's ~2.3us per add serializes all
            # seven xsums, and xsum[6] gates the block's first C group
            eng = nc.gpsimd if f % 2 == 0 else nc.vector
            eng.tensor_add(xs_, x2s[0][f], x2s[1][f])
            xsum[f] = xs_

        def segs_of(tb, k):
            """Column segments (er_chunk, c0, c1) covering the block. The
            host sub-sorts tokens inside each block by (slot0-chunk,
            slot1-chunk), so every token column is contracted against ONLY
            its own er-chunk — the LoRA matmul work per column halves vs
            contracting both chunks everywhere."""
            if spec is None:
                return ((0, 0, TBLK), (1, 0, TBLK))
            return spec[tb][k]

        def emit_B_f(tb, k, f, base1, base3, rws, m1, m3, x2s, xsum,
                     do_xsum=True, altps=False):
            """LoRA up-proj + silu/mul chain for one (slot, f-chunk).
            altps (last block): alternate the PSUM tags between pspB's
            qA/qB and psp's idle pA/pB so each tag is reused every OTHER
            iteration — the DVE chain then never gates the leader matmul."""
            segs = segs_of(tb, k)
            fsl = slice(f * 128, (f + 1) * 128)
            if altps and f % 2 == 1:
                psA = psp.tile([128, TBLK], F32, name="psA", tag="pA")
                psB = psp.tile([128, TBLK], F32, name="psB", tag="pB")
            else:
                psA = pspB.tile([128, TBLK], F32, name="psA", tag="qA")
                psB = pspB.tile([128, TBLK], F32, name="psB", tag="qB")
            for (er, c0, c1) in segs:
                nc.tensor.matmul(psA[:, c0:c1], b1s[:, er, fsl],
                                 m1[k][:, er, c0:c1], start=True, stop=True)
            for (er, c0, c1) in segs:
                nc.tensor.matmul(psB[:, c0:c1], b3s[:, er, fsl],
                                 m3[k][:, er, c0:c1], start=True, stop=True)
            t1_ = trans.tile([128, TBLK], BF16, name="t1", tag="t1")
            nc.vector.tensor_add(t1_, psA, base1[f])
            sl_ = trans.tile([128, TBLK], BF16, name="sl", tag="sl")
            nc.scalar.activation(sl_, t1_, mybir.ActivationFunctionType.Silu,
                                 scale=1.0 / SCALE)
            t3_ = trans.tile([128, TBLK], BF16, name="t3", tag="t3")
            nc.vector.tensor_add(t3_, psB, base3[f])
            x3s_ = trans.tile([128, TBLK], BF16, name="x3s", tag="x3s")
            nc.vector.tensor_mul(x3s_, t3_, rws[k])
            x2_ = actp.tile([128, TBLK], BF16, name=f"x2_{k}{f}",
                            tag=f"x2_{k}{f}")
            nc.vector.tensor_mul(x2_, sl_, x3s_)
            x2s[k][f] = x2_
            if k == K - 1 and do_xsum:
                emit_xsum(f, x2s, xsum)

        def emit_a2(tb, k, x2s):
            t0 = tb * TBLK
            segs = segs_of(tb, k)
            for er in range(NER):
                ranges = [(c0, c1) for (e, c0, c1) in segs if e == er]
                if not ranges:
                    continue
                ers = slice(er * 128, (er + 1) * 128)
                psa2 = pspD.tile([128, TBLK], F32, name="psa2", tag="pD")
                for (c0, c1) in ranges:
                    for f in range(NF):
                        nc.tensor.matmul(psa2[:, c0:c1], a2s[f][:, ers],
                                         x2s[k][f][:, c0:c1],
                                         start=(f == 0), stop=(f == NF - 1))
                m2_ = actp.tile([128, TBLK], BF16, name=f"m2_{k}{er}",
                                tag=f"m2_{k}{er}")
                eng = nc.scalar if (k + er) % 2 == 0 else nc.sync
                # copy/DMA only the covered ranges: the uncovered psa2
                # columns are stale PSUM (m2o is zero-initialized and the
                # host masks per-token, so untouched regions contribute 0).
                for (c0, c1) in ranges:
                    nc.scalar.copy(m2_[:, c0:c1], psa2[:, c0:c1])
                    eng.dma_start(out=m2o[k][er][:, t0 + c0:t0 + c1],
                                  in_=m2_[:, c0:c1])

        def emit_C_group(tb, xsum, h, lastblk=False):
            tsl = slice(tb * TBLK, (tb + 1) * TBLK)
            hsl = slice(h * 128, (h + 1) * 128)
            if lastblk:
                # pA/pB belong to the last block's Bx iterations there
                psD = pspD.tile([128, TBLK], F32, name="psD", tag="pD")
            else:
                # ride the phase-A banks (idle once A'(5)/A'(6) have been
                # copied out): double-buffered leaders instead of sharing
                # pD's 2 banks with a2
                psD = psp.tile([128, TBLK], F32, name="psD",
                               tag=("pA" if h % 2 == 0 else "pB"))
            for f in range(NF):
                nc.tensor.matmul(psD, w2s[f][:, hsl], xsum[f],
                                 start=(f == 0), stop=(f == NF - 1))
            o_ = outp.tile([128, TBLK], BF16, name="osb", tag="osb")
            nc.scalar.copy(o_, psD)
            eng = nc.sync if h % 2 == 0 else nc.scalar
            eng.dma_start(out=outT[h][:, tsl], in_=o_)

        def emit_C_group_split(tb, xsum, h):
            """Very last C group, column-split with TWO half-bank PSUM
            tiles so half-1's copy+DMA overlap half-2's matmuls (a single
            tile serializes on Tile's per-tile read/write tracking)."""
            t0 = tb * TBLK
            hsl = slice(h * 128, (h + 1) * 128)
            HB = TBLK // 2
            for i, c0 in enumerate((0, HB)):
                psD = pspD.tile([128, HB], F32, name=f"psDs{i}", tag="pD")
                for f in range(NF):
                    nc.tensor.matmul(psD, w2s[f][:, hsl],
                                     xsum[f][:, c0:c0 + HB],
                                     start=(f == 0), stop=(f == NF - 1))
                o_ = outp.tile([128, HB], BF16, name="osbs", tag="osb")
                nc.scalar.copy(o_, psD)
                eng = nc.sync if i == 0 else nc.scalar
                eng.dma_start(out=outT[h][:, t0 + c0:t0 + c0 + HB], in_=o_)

        def emit_C(tb, xsum, last=False):
            for h in range(NH):
                if last and h == NH - 1:
                    emit_C_group_split(tb, xsum, h)
                else:
                    emit_C_group(tb, xsum, h, lastblk=last)

        # ---- software pipeline: next block's phase-A groups are woven
        # BETWEEN this block's phase-B iterations (PE executes in emission
        # order, so independent work must be emitted before gated work).
        # Phase B is Vector-throughput-bound (~2.2us of DVE chain per
        # f-iteration); each interposed A group gives the DVE ~7us of
        # matmul cover to drain its chain backlog, so the B-group PSUM
        # leaders never wait on bank release. ----
        xs, rws, m1, m3 = pre0
        base1, base3 = emit_A(xs)
        heldC = None   # xsum of block NT-2, its C woven into the last block
        for tb in range(NT):
            x2s = [[None] * NF for _ in range(K)]
            xsum = [None] * NF
            Bf = lambda k, f: emit_B_f(tb, k, f, base1, base3, rws, m1, m3,
                                       x2s, xsum)
            if tb + 1 < NT:
                xsn, rwsn, m1n, m3n = load_block_inputs(tb + 1)
                b1n, b3n = [None] * NF, [None] * NF
                A = lambda f: emit_A_group(xsn, f, b1n, b3n)
                Bf(0, 0); Bf(0, 1)
                Bf(0, 2); Bf(0, 3); A(0)
                Bf(0, 4); Bf(0, 5); A(1)
                Bf(0, 6); Bf(1, 0); A(2)
                Bf(1, 1); Bf(1, 2); A(3)
                Bf(1, 3); Bf(1, 4); emit_a2(tb, 0, x2s)
                Bf(1, 5); Bf(1, 6); A(4)
                emit_a2(tb, 1, x2s)
                A(5); A(6)
                xs, rws, m1, m3 = xsn, rwsn, m1n, m3n
                base1, base3 = b1n, b3n
                if tb == NT - 2:
                    heldC = xsum      # defer C(NT-2) into the last block
                else:
                    emit_C(tb, xsum)
            else:
                # last block has no next-A cover; weave the held-back
                # C(NT-2) groups among the k=0 iterations instead. All held
                # groups must be emitted before B(1,0) writes xsum (the
                # single-buffered xsum tags roll over to this block there).
                hq = list(range(NH))
                C2 = lambda n: [emit_C_group(tb - 1, heldC, hq.pop(0),
                                             lastblk=True)
                                for _ in range(n)]
                Bx = lambda k, f: emit_B_f(tb, k, f, base1, base3, rws, m1,
                                           m3, x2s, xsum, do_xsum=False,
                                           altps=True)
                Bx(0, 0); Bx(1, 0); C2(3)
                Bx(0, 1); Bx(1, 1); C2(3)
                Bx(0, 2); Bx(1, 2); C2(2)
                Bx(0, 3); Bx(1, 3); C2(2)
                Bx(0, 4); Bx(1, 4); C2(2)
                Bx(0, 5); Bx(1, 5); C2(2)
                Bx(0, 6); Bx(1, 6); C2(2)
                for f in range(NF):
                    emit_xsum(f, x2s, xsum)
                emit_a2(tb, 0, x2s)
                emit_a2(tb, 1, x2s)
                emit_C(tb, xsum, last=True)

        if loop_cm is not None:
            loop_cm.__exit__(None, None, None)

    nc.finalize()
    return nc


def _q8(a, scale):
    return np.clip(a * scale, -240.0, 240.0).astype(NPF8E4)


def prepare_inputs(hidden_states, Wg, W1, W2, W3, A1, B1, A2, B2, A3, B3):
    """Host preprocessing: routing + per-core weight slicing/casting."""
    hidden_states, Wg, W1, W2, W3, A1, B1, A2, B2, A3, B3 = (
        np.asarray(a, dtype=np.float32)
        for a in (hidden_states, Wg, W1, W2, W3, A1, B1, A2, B2, A3, B3))
    x = np.ascontiguousarray(hidden_states.reshape(T, H))

    logits = x @ Wg.T.astype(np.float32)
    m = logits.max(-1, keepdims=True)
    p = np.exp(logits - m, dtype=np.float32)
    p /= p.sum(-1, keepdims=True)
    sel = np.argsort(-p, axis=-1, kind="stable")[:, :K]      # [T, K]
    rw = np.take_along_axis(p, sel, axis=1)
    rw = (rw / rw.sum(-1, keepdims=True)).astype(np.float32)  # [T, K]

    # Sort tokens by slot-0 EXPERT (block composition), then inside each
    # block by (slot0-chunk, slot1-chunk). Each block then decomposes into
    # a few contiguous column segments per slot, each touching a single
    # er-chunk — the device contracts every token column against only ITS
    # chunk instead of both.
    GE = E // NER                         # experts per er-chunk
    perm = np.argsort(sel[:, 0], kind="stable")
    for b in range(NT):
        idx = perm[b * TBLK:(b + 1) * TBLK]
        key = (sel[idx, 0] // GE) * NER + (sel[idx, 1] // GE)
        perm[b * TBLK:(b + 1) * TBLK] = idx[np.argsort(key, kind="stable")]
    x = np.ascontiguousarray(x[perm])
    sel = sel[perm]
    rw = np.ascontiguousarray(rw[perm])

    spec = []
    for b in range(NT):
        per_slot = []
        for k in range(K):
            ch = sel[b * TBLK:(b + 1) * TBLK, k] // GE
            segs, start = [], 0
            for i in range(1, TBLK + 1):
                if i == TBLK or ch[i] != ch[i - 1]:
                    segs.append((int(ch[start]), start, i))
                    start = i
            per_slot.append(tuple(segs))
        spec.append(tuple(per_slot))
    spec = tuple(spec)

    xT_np = np.ascontiguousarray(
        x.T.reshape(NH // 2, 2, 128, T).transpose(0, 2, 1, 3)
    ).astype(NPBF16)                                  # [NH//2, 128, 2, T]

    # per-slot one-hot masks over the (e, r) axis, transposed to [ER, T];
    # applied HOST-side to the returned a2 (masking is elementwise, so it
    # commutes with the cross-core partial sum)
    masks = np.zeros((K, ER, T), dtype=np.float32)
    for k in range(K):
        onehot = np.zeros((T, E), np.float32)
        onehot[np.arange(T), sel[:, k]] = 1.0
        masks[k] = np.repeat(onehot, R, axis=1).T
    # rw is pre-divided by SCALE: the device's x3 path multiplies the
    # SCALE-scaled (base3 + lora3) PSUM values by it, landing on true scale.
    rwr_np = np.ascontiguousarray(rw.T / SCALE).reshape(K, 1, T).astype(NPBF16)

    # flattened LoRA tensors (full copies; small)
    A1f = A1.reshape(ER, H)                      # [er, H]
    A3f = A3.reshape(ER, H)
    B2f = B2.transpose(0, 2, 1).reshape(ER, H)   # [er, H]

    # per-slot masked LoRA down-projections, computed host-side in fp32,
    # quantized to fp8e4 (x S_M) in the DoubleRow [Ki=128, Ko=NER, T] layout
    a1_all = x @ A1f.T.astype(np.float32)        # [T, ER]
    a3_all = x @ A3f.T.astype(np.float32)
    m1t_np = np.zeros((K, ER, T), dtype=NPF8E4)
    m3t_np = np.zeros((K, ER, T), dtype=NPF8E4)
    for k in range(K):
        mx = np.repeat(
            np.eye(E, dtype=np.float32)[sel[:, k]], R, axis=1)   # [T, ER]
        m1t_np[k] = _q8((a1_all * mx).T, S_M)
        m3t_np[k] = _q8((a3_all * mx).T, S_M)
    m1t_np = np.ascontiguousarray(
        m1t_np.reshape(K, NER, 128, T).transpose(0, 2, 1, 3))
    m3t_np = np.ascontiguousarray(
        m3t_np.reshape(K, NER, 128, T).transpose(0, 2, 1, 3))

    def pack_fmajor(wT):
        # [FS, H] -> [NF, 128, NH, 128]: [f, k, hc, m] = W[f*128+m, hc*128+k]
        return np.ascontiguousarray(
            wT.reshape(NF, 128, NH, 128).transpose(0, 3, 2, 1))

    in_maps = []
    for c in range(NCORES):
        fs = slice(c * FS, (c + 1) * FS)
        w1t_np = pack_fmajor(W1[fs]).astype(NPBF16)
        w3t_np = pack_fmajor(W3[fs]).astype(NPBF16)
        w2T = np.ascontiguousarray(W2[:, fs].T).astype(NPBF16)  # [FS, H]
        w2t_np = w2T.reshape(NF, 128, H)
        b1f = B1[:, fs, :].transpose(0, 2, 1).reshape(ER, FS)   # [er, f]
        b3f = B3[:, fs, :].transpose(0, 2, 1).reshape(ER, FS)
        b1t_np = np.ascontiguousarray(
            _q8(b1f, S_B).reshape(NER, 128, FS).transpose(1, 0, 2))
        b3t_np = np.ascontiguousarray(
            _q8(b3f, S_B).reshape(NER, 128, FS).transpose(1, 0, 2))
        a2f = A2[:, :, fs].reshape(ER, FS)                      # [er, f]
        a2t_np = np.ascontiguousarray(a2f.T).astype(NPBF16).reshape(NF, 128, ER)

        in_maps.append({
            "xT": xT_np, "w1t": w1t_np, "w3t": w3t_np, "w2t": w2t_np,
            "m1t": m1t_np, "m3t": m3t_np, "b1t": b1t_np, "b3t": b3t_np,
            "a2t": a2t_np,
            "rwr": rwr_np,
        })
    return in_maps, (B2f.astype(np.float32), masks, perm, spec)


_CACHED_NC = {}


def kernel(hidden_states, Wg, W1, W2, W3, A1, B1, A2, B2, A3, B3,
           _trace=False, _tmpdir=None):
    in_maps, (B2f, masks, perm, spec) = prepare_inputs(
        hidden_states, Wg, W1, W2, W3, A1, B1, A2, B2, A3, B3)
    if spec not in _CACHED_NC:
        _CACHED_NC[spec] = build_nc(spec=spec)
    nc = _CACHED_NC[spec]
    res = run_bass_kernel_spmd(nc, in_maps, list(range(NCORES)),
                               trace=_trace, tmpdir=_tmpdir)
    acc = np.zeros((NH, 128, T), np.float32)
    m2sum = np.zeros((K, ER, T), np.float32)
    for c in range(NCORES):
        acc += res.results[c]["outT"].astype(np.float32)
        m2sum += res.results[c]["m2o"].reshape(K, ER, T).astype(np.float32)
    out = acc.reshape(H, T)
    # host-side lora2: mask the (unmasked, core-summed) a2, then the final
    # LoRA up-projection is linear -> one small GEMM per slot
    for k in range(K):
        out += B2f.T @ (m2sum[k] * masks[k])
    outT_tok = out.T                       # [T, H], token-permuted order
    final = np.empty_like(outT_tok)
    final[perm] = outT_tok                 # undo the expert sort
    out = final.reshape(B, S, H)
    kernel.last_results = res
    return out


if __name__ == "__main__":
    nc = build_nc(spec=None)
    print("built ok")


# revision 40
# speedup vs baseline: 1.0085x; 1.0006x over previous
"""Mixtral sparse-MoE block with per-expert LoRA adapters on 8 Trainium2 cores.

Problem shapes: B=2, S=1024, H=2048, F=7168, E=8, R=32, top-K=2.
T = B*S = 2048 tokens.

Sharding: tensor-parallel over the FFN dim F. Core c owns rows
[c*896:(c+1)*896] of W1/W3 (and the matching B1/B3 LoRA rows) and the same
columns of W2/A2. Everything after the silu is linear in
x2s = silu(x1)*x3*rw, so each core emits an exact partial [H, T] output over
its F-shard (bf16) and the host sums the 8 partials in fp32.

Work split (device vs host):
- Host: gating (softmax + top-2), the tiny per-expert LoRA down-projections
  a1/a3 = x @ A{1,3}T masked per slot (m1/m3 uploads, fp8e4), and the final
  LoRA up-projection lora2 = B2 @ sum_cores(m2) (one small GEMM).
- Device (per core): base1/base3 = x @ W{1,3}[shard].T, the per-slot LoRA
  up-projections lora1/3 = B{1,3}[shard] @ m{1,3} accumulated in PSUM,
  silu/mul/scale chain, a2 = A2[shard] @ x2s per slot (m2, returned to
  host), and the down-projection on the slot-summed activations.

Perf structure (measured on trn2 via NTFF hardware traces):
- PE-bound: a dense bf16 MM stream runs ~216ns per K=128/N=512 matmul at
  2.4GHz. Reductions vs the v1 kernel:
  * w1/w3 are streamed F-CHUNK-major ([NF,128,NH,128] host packs): block-0
    phase A consumes weights in DMA arrival order, so the PE starts ~5us
    earlier and the ~7us of warmup stalls (waiting for h-major pair DMAs)
    disappear.
  * Tokens are sorted by slot-0 expert, then inside each block by
    (slot0-chunk, slot1-chunk). Each block decomposes into contiguous
    column segments per slot that touch a single er-chunk, so the LoRA
    up-projections and a2 contract every token column against only ITS
    er-chunk: ~half the LoRA matmul columns vs dense. (fp8 DoubleRow was
    tried for the dense case and measured at only ~6% over two bf16
    matmuls — column splitting beats it and needs no perf mode.)
  * m1/m3/b1/b3 are host-quantized fp8e4 with power-of-2 scales (S_M=16,
    S_B=1024); the product scale 1/16384 is folded into the silu's input
    scale and the host-prescaled rw tensor — zero dequant ops on device.
  * outT is written bf16 (halves the output drain) and output DMAs
    alternate between the sync/scalar HWDGE rings.
  * Phase-A PSUM->SBUF copies run on the DVE, not the scalar engine: the
    scalar engine's stream is clogged by DMA-issue instructions during
    warmup (each blocks on HWDGE queue space), which held PSUM banks
    hostage and stalled phase A for ~17us.
- The block loop is software-pipelined: block b+1's phase-A groups are
  EMITTED between block b's phase-B iterations. Phase B is DVE-bound
  (~2.2us of add/silu/mul chain per f-iteration), and the PE executes in
  emission order, so each interposed phase-A group gives the DVE a ~7us
  matmul window to drain its backlog. The last block (no next A) instead
  weaves in the held-back phase-C groups of block NT-2.
- Every dma_start costs ~0.6us of ISSUE time on its HWDGE ring regardless
  of size; streamed tensors are host-packed so one DMA feeds multiple SBUF
  chunk views. Only sync/scalar HWDGE rings are used.
- Outputs are unmasked a2 partials; the (elementwise) expert mask commutes
  with the cross-core sum and is applied on host before the lora2 GEMM.
"""

import sys
from contextlib import ExitStack

import numpy as np

try:
    import concourse.bass as bass  # noqa: F401
except ImportError:
    sys.path.insert(0, "/opt/trn_rl_repo")

import ml_dtypes

import concourse.bass as bass
import concourse.mybir as mybir
import concourse.tile as tile
from concourse import bacc
from concourse.bass_utils import run_bass_kernel_spmd

BF16 = mybir.dt.bfloat16
F8E4 = mybir.dt.float8e4
F32 = mybir.dt.float32
NPBF16 = ml_dtypes.bfloat16
NPF8E4 = ml_dtypes.float8_e4m3

B, S, H, F, E, R, K = 2, 1024, 2048, 7168, 8, 32, 2
T = B * S                      # 2048 tokens
ER = E * R                     # 256
NCORES = 8
FS = F // NCORES               # 896 per-core F shard
NH = H // 128                  # 16 h-chunks
NF = FS // 128                 # 7 f-chunks (per core)
NER = ER // 128                # 2 er-chunks
TBLK = 512
NT = T // TBLK                 # 4 token blocks

S_M = 16.0                     # fp8 scale on m1/m3 (a-values, sigma ~0.9)
S_B = 1024.0                   # fp8 scale on b1/b3 (weights, sigma ~0.02)
SCALE = S_M * S_B              # lora PSUM scale; folded into silu + rw


def build_nc(repeat=None, spec=None):
    """Build the per-core Bass module.

    spec: per-block tuple of active er-chunks for slot 0 (from the host's
    exact-expert token sort); None means dense (0, 1) everywhere.
    """
    nc = bacc.Bacc(None)

    # x is host-packed in h-chunk pairs [NH//2, 128, 2, T]; one DMA feeds
    # two SBUF chunk-views.
    xT = nc.declare_dram_parameter("xT", [NH // 2, 128, 2, T], BF16, isOutput=False)
    # w1/w3 are host-packed F-CHUNK-major: [NF, 128, NH, 128] — w1t[f][k,hc,m]
    # = W1[shard_f*128+m, hc*128+k]. One DMA per f-chunk means block-0's
    # phase A consumes weights in arrival order (A(f) needs only chunk f).
    w1t = nc.declare_dram_parameter("w1t", [NF, 128, NH, 128], BF16, isOutput=False)
    w3t = nc.declare_dram_parameter("w3t", [NF, 128, NH, 128], BF16, isOutput=False)
    w2t = nc.declare_dram_parameter("w2t", [NF, 128, H], BF16, isOutput=False)
    # fp8 LoRA operands: m{1,3} [K, 128, NER, T] (x S_M), b{1,3} [128, NER, FS]
    # (x S_B). Layout matches DoubleRow's [Ki, Ko=2, dim] AP: partition k is
    # er-row c*128+k of chunk c.
    m1t = nc.declare_dram_parameter("m1t", [K, 128, NER, T], F8E4, isOutput=False)
    m3t = nc.declare_dram_parameter("m3t", [K, 128, NER, T], F8E4, isOutput=False)
    b1t = nc.declare_dram_parameter("b1t", [128, NER, FS], F8E4, isOutput=False)
    b3t = nc.declare_dram_parameter("b3t", [128, NER, FS], F8E4, isOutput=False)
    a2t = nc.declare_dram_parameter("a2t", [NF, 128, ER], BF16, isOutput=False)
    rwr = nc.declare_dram_parameter("rwr", [K, 1, T], BF16, isOutput=False)
    outT = nc.declare_dram_parameter("outT", [NH, 128, T], BF16, isOutput=True)
    m2o = nc.declare_dram_parameter("m2o", [K, NER, 128, T], BF16, isOutput=True)

    with tile.TileContext(nc) as tc, ExitStack() as ctx:
        resw = ctx.enter_context(tc.tile_pool(name="resw", bufs=1))
        xsp = ctx.enter_context(tc.tile_pool(name="xsp", bufs=2))
        actp = ctx.enter_context(tc.tile_pool(name="actp", bufs=1))
        mp_ = ctx.enter_context(tc.tile_pool(name="mp", bufs=2))
        trans = ctx.enter_context(tc.tile_pool(name="trans", bufs=4))
        outp = ctx.enter_context(tc.tile_pool(name="outp", bufs=4))
        # PSUM partition: phase A gets 4 banks (2 tags x 2 bufs), phase B's
        # short LoRA groups 2 banks, a2 + down-proj share 2 banks.
        psp = ctx.enter_context(tc.tile_pool(name="psp", bufs=2, space="PSUM"))
        pspB = ctx.enter_context(tc.tile_pool(name="pspB", bufs=1, space="PSUM"))
        pspD = ctx.enter_context(tc.tile_pool(name="pspD", bufs=2, space="PSUM"))

        loop_cm = tc.For_i(0, repeat, 1) if repeat is not None else None
        if loop_cm is not None:
            loop_cm.__enter__()

        # ---- per-block input streamers ----
        def load_block_inputs(tb, xs=None):
            tsl = slice(tb * TBLK, (tb + 1) * TBLK)
            if xs is None:
                xs = []
                for hp in range(NH // 2):
                    xt_ = xsp.tile([128, 2 * TBLK], BF16, name=f"x{hp}",
                                   tag=f"x{hp}")
                    nc.sync.dma_start(out=xt_, in_=xT[hp][:, :, tsl])
                    xs.append(xt_[:, 0:TBLK])
                    xs.append(xt_[:, TBLK:2 * TBLK])
            m1, m3 = [None] * K, [None] * K
            for k in range(K):
                m1_ = mp_.tile([128, NER, TBLK], F8E4, name=f"m1_{k}",
                               tag=f"m1_{k}")
                nc.sync.dma_start(out=m1_, in_=m1t[k][:, :, tsl])
                m1[k] = m1_
                m3_ = mp_.tile([128, NER, TBLK], F8E4, name=f"m3_{k}",
                               tag=f"m3_{k}")
                nc.scalar.dma_start(out=m3_, in_=m3t[k][:, :, tsl])
                m3[k] = m3_
            rws = []
            for k in range(K):
                r_ = mp_.tile([128, TBLK], BF16, name=f"rw{k}", tag=f"rw{k}")
                nc.sync.dma_start(out=r_, in_=rwr[k][:, tsl].to_broadcast([128, TBLK]))
                rws.append(r_)
            return xs, rws, m1, m3

        # ---- resident weights, emitted in CONSUMPTION order so the two
        # ~170GB/s HWDGE rings deliver each tensor just before phase A of
        # block 0 needs it: f0 (split in h-halves for the earliest first
        # matmul) interleaved with the first x pairs, then f1..f6, then the
        # lora/m inputs, then w2/a2 (needed ~60us in). ----
        w1f, w3f = [None] * NF, [None] * NF
        NQ = NH // 4
        w1f0, w3f0 = [None] * 4, [None] * 4   # h-quarters of f-chunk 0

        def load_wf0_quarter(i):
            hs = slice(i * NQ, (i + 1) * NQ)
            t1 = resw.tile([128, NQ, 128], BF16, name=f"w1f0{i}",
                           tag=f"w1f0{i}")
            nc.sync.dma_start(out=t1, in_=w1t[0][:, hs, :])
            w1f0[i] = t1
            t3 = resw.tile([128, NQ, 128], BF16, name=f"w3f0{i}",
                           tag=f"w3f0{i}")
            nc.scalar.dma_start(out=t3, in_=w3t[0][:, hs, :])
            w3f0[i] = t3

        def load_wf(f):
            t1 = resw.tile([128, NH, 128], BF16, name=f"w1f{f}", tag=f"w1f{f}")
            nc.sync.dma_start(out=t1, in_=w1t[f])
            w1f[f] = t1
            t3 = resw.tile([128, NH, 128], BF16, name=f"w3f{f}", tag=f"w3f{f}")
            nc.scalar.dma_start(out=t3, in_=w3t[f])
            w3f[f] = t3

        # f-chunk 1 splits each tensor's h-halves ACROSS the rings: the
        # first halves land ~1.5us earlier than a single-ring full chunk,
        # which is exactly the margin A(1)'s leader was stalling on.
        w1f1h, w3f1h = [None] * 2, [None] * 2

        def load_wf1_split():
            ha = slice(0, NH // 2)
            hb = slice(NH // 2, NH)
            t1a = resw.tile([128, NH // 2, 128], BF16, name="w1f1a", tag="w1f1a")
            nc.sync.dma_start(out=t1a, in_=w1t[1][:, ha, :])
            w1f1h[0] = t1a
            t3a = resw.tile([128, NH // 2, 128], BF16, name="w3f1a", tag="w3f1a")
            nc.scalar.dma_start(out=t3a, in_=w3t[1][:, ha, :])
            w3f1h[0] = t3a
            t1b = resw.tile([128, NH // 2, 128], BF16, name="w1f1b", tag="w1f1b")
            nc.scalar.dma_start(out=t1b, in_=w1t[1][:, hb, :])
            w1f1h[1] = t1b
            t3b = resw.tile([128, NH // 2, 128], BF16, name="w3f1b", tag="w3f1b")
            nc.sync.dma_start(out=t3b, in_=w3t[1][:, hb, :])
            w3f1h[1] = t3b

        def w1v(f, h):
            if f == 0:
                return w1f0[h // NQ][:, h % NQ, :]
            if f == 1:
                return w1f1h[h // (NH // 2)][:, h % (NH // 2), :]
            return w1f[f][:, h, :]

        def w3v(f, h):
            if f == 0:
                return w3f0[h // NQ][:, h % NQ, :]
            if f == 1:
                return w3f1h[h // (NH // 2)][:, h % (NH // 2), :]
            return w3f[f][:, h, :]

        xs0 = []

        def load_x0_pair(hp):
            xt_ = xsp.tile([128, 2 * TBLK], BF16, name=f"x{hp}", tag=f"x{hp}")
            xeng = nc.sync if hp % 2 == 0 else nc.scalar
            xeng.dma_start(out=xt_, in_=xT[hp][:, :, 0:TBLK])
            xs0.append(xt_[:, 0:TBLK])
            xs0.append(xt_[:, TBLK:2 * TBLK])

        load_wf0_quarter(0)
        load_x0_pair(0); load_x0_pair(1)
        load_wf0_quarter(1)
        load_x0_pair(2); load_x0_pair(3)
        load_wf0_quarter(2)
        load_x0_pair(4); load_x0_pair(5)
        load_wf0_quarter(3)
        load_x0_pair(6); load_x0_pair(7)
        load_wf1_split()
        for f in range(2, NF):
            load_wf(f)
        b1s = resw.tile([128, NER, FS], F8E4, name="b1s", tag="b1s")
        nc.sync.dma_start(out=b1s, in_=b1t[:, :, :])
        b3s = resw.tile([128, NER, FS], F8E4, name="b3s", tag="b3s")
        nc.scalar.dma_start(out=b3s, in_=b3t[:, :, :])
        pre0 = load_block_inputs(0, xs0)
        w2s, a2s = [], []
        for f in range(NF):
            eng = nc.sync if f % 2 == 0 else nc.scalar
            t_ = resw.tile([128, H], BF16, name=f"w2s{f}", tag=f"w2s{f}")
            eng.dma_start(out=t_, in_=w2t[f])
            w2s.append(t_)
        for f in range(NF):
            eng = nc.scalar if f % 2 == 0 else nc.sync
            t_ = resw.tile([128, ER], BF16, name=f"a2s{f}", tag=f"a2s{f}")
            eng.dma_start(out=t_, in_=a2t[f])
            a2s.append(t_)

        # ---- phase emitters (software-pipelined across blocks below) ----
        def emit_A_group(xs, f, base1, base3):
            """One f-chunk of base1/base3 = W1/W3 @ x (PE-dense, no deps).
            The PSUM->SBUF copies scale by SCALE so phase B's adds work in
            the fp8-product scale with zero extra ops. The last f-chunk
            rides the pD banks (idle right after a2) so the block's first
            C-group leader on pA waits one A-copy less in the DVE queue."""
            if f == NF - 1:
                ps1 = pspD.tile([128, TBLK], F32, name="ps1", tag="pD")
                ps3 = pspD.tile([128, TBLK], F32, name="ps3", tag="pD")
            else:
                ps1 = psp.tile([128, TBLK], F32, name="ps1", tag="pA")
                ps3 = psp.tile([128, TBLK], F32, name="ps3", tag="pB")
            for h in range(NH):
                nc.tensor.matmul(ps1, w1v(f, h), xs[h], start=(h == 0), stop=(h == NH - 1))
                nc.tensor.matmul(ps3, w3v(f, h), xs[h], start=(h == 0), stop=(h == NH - 1))
            # copies ride the DVE: the scalar engine's stream is clogged by
            # DMA-issue instructions early on (queue-full waits), and a
            # scalar copy here would delay the PSUM bank release that gates
            # the next A group's leader matmul.
            b1_ = actp.tile([128, TBLK], BF16, name=f"b1_{f}", tag=f"b1_{f}")
            nc.vector.tensor_scalar_mul(b1_, ps1, SCALE)
            base1[f] = b1_
            b3_ = actp.tile([128, TBLK], BF16, name=f"b3_{f}", tag=f"b3_{f}")
            nc.vector.tensor_scalar_mul(b3_, ps3, SCALE)
            base3[f] = b3_

        def emit_A(xs):
            base1, base3 = [None] * NF, [None] * NF
            for f in range(NF):
                emit_A_group(xs, f, base1, base3)
            return base1, base3

        def emit_xsum(f, x2s, xsum):
            xs_ = actp.tile([128, TBLK], BF16, name=f"xsum{f}",
                            tag=f"xsum{f}")
            # alternate gpsimd/DVE: gpsimd's ~2.3us per add serializes all
            # seven xsums, and xsum[6] gates the block's first C group
            eng = nc.gpsimd if f % 2 == 0 else nc.vector
            eng.tensor_add(xs_, x2s[0][f], x2s[1][f])
            xsum[f] = xs_

        def segs_of(tb, k):
            """Column segments (er_chunk, c0, c1) covering the block. The
            host sub-sorts tokens inside each block by (slot0-chunk,
            slot1-chunk), so every token column is contracted against ONLY
            its own er-chunk — the LoRA matmul work per column halves vs
            contracting both chunks everywhere."""
            if spec is None:
                return ((0, 0, TBLK), (1, 0, TBLK))
            return spec[tb][k]

        def emit_B_f(tb, k, f, base1, base3, rws, m1, m3, x2s, xsum,
                     do_xsum=True, altps=False):
            """LoRA up-proj + silu/mul chain for one (slot, f-chunk).
            altps (last block): alternate the PSUM tags between pspB's
            qA/qB and psp's idle pA/pB so each tag is reused every OTHER
            iteration — the DVE chain then never gates the leader matmul."""
            segs = segs_of(tb, k)
            fsl = slice(f * 128, (f + 1) * 128)
            if altps and f % 2 == 1:
                psA = psp.tile([128, TBLK], F32, name="psA", tag="pA")
                psB = psp.tile([128, TBLK], F32, name="psB", tag="pB")
            else:
                psA = pspB.tile([128, TBLK], F32, name="psA", tag="qA")
                psB = pspB.tile([128, TBLK], F32, name="psB", tag="qB")
            for (er, c0, c1) in segs:
                nc.tensor.matmul(psA[:, c0:c1], b1s[:, er, fsl],
                                 m1[k][:, er, c0:c1], start=True, stop=True)
            for (er, c0, c1) in segs:
                nc.tensor.matmul(psB[:, c0:c1], b3s[:, er, fsl],
                                 m3[k][:, er, c0:c1], start=True, stop=True)
            t1_ = trans.tile([128, TBLK], BF16, name="t1", tag="t1")
            nc.vector.tensor_add(t1_, psA, base1[f])
            sl_ = trans.tile([128, TBLK], BF16, name="sl", tag="sl")
            nc.scalar.activation(sl_, t1_, mybir.ActivationFunctionType.Silu,
                                 scale=1.0 / SCALE)
            t3_ = trans.tile([128, TBLK], BF16, name="t3", tag="t3")
            nc.vector.tensor_add(t3_, psB, base3[f])
            x3s_ = trans.tile([128, TBLK], BF16, name="x3s", tag="x3s")
            nc.vector.tensor_mul(x3s_, t3_, rws[k])
            x2_ = actp.tile([128, TBLK], BF16, name=f"x2_{k}{f}",
                            tag=f"x2_{k}{f}")
            nc.vector.tensor_mul(x2_, sl_, x3s_)
            x2s[k][f] = x2_
            if k == K - 1 and do_xsum:
                emit_xsum(f, x2s, xsum)

        def emit_a2(tb, k, x2s):
            t0 = tb * TBLK
            segs = segs_of(tb, k)
            for er in range(NER):
                ranges = [(c0, c1) for (e, c0, c1) in segs if e == er]
                if not ranges:
                    continue
                ers = slice(er * 128, (er + 1) * 128)
                psa2 = pspD.tile([128, TBLK], F32, name="psa2", tag="pD")
                for (c0, c1) in ranges:
                    for f in range(NF):
                        nc.tensor.matmul(psa2[:, c0:c1], a2s[f][:, ers],
                                         x2s[k][f][:, c0:c1],
                                         start=(f == 0), stop=(f == NF - 1))
                m2_ = actp.tile([128, TBLK], BF16, name=f"m2_{k}{er}",
                                tag=f"m2_{k}{er}")
                eng = nc.scalar if (k + er) % 2 == 0 else nc.sync
                # copy/DMA only the covered ranges: the uncovered psa2
                # columns are stale PSUM (m2o is zero-initialized and the
                # host masks per-token, so untouched regions contribute 0).
                for (c0, c1) in ranges:
                    nc.scalar.copy(m2_[:, c0:c1], psa2[:, c0:c1])
                    eng.dma_start(out=m2o[k][er][:, t0 + c0:t0 + c1],
                                  in_=m2_[:, c0:c1])

        def emit_C_group(tb, xsum, h, lastblk=False):
            tsl = slice(tb * TBLK, (tb + 1) * TBLK)
            hsl = slice(h * 128, (h + 1) * 128)
            if lastblk:
                # pA/pB belong to the last block's Bx iterations there
                psD = pspD.tile([128, TBLK], F32, name="psD", tag="pD")
            else:
                # ride the phase-A banks (idle once A'(5)/A'(6) have been
                # copied out): double-buffered leaders instead of sharing
                # pD's 2 banks with a2
                psD = psp.tile([128, TBLK], F32, name="psD",
                               tag=("pA" if h % 2 == 0 else "pB"))
            for f in range(NF):
                nc.tensor.matmul(psD, w2s[f][:, hsl], xsum[f],
                                 start=(f == 0), stop=(f == NF - 1))
            o_ = outp.tile([128, TBLK], BF16, name="osb", tag="osb")
            nc.scalar.copy(o_, psD)
            eng = nc.sync if h % 2 == 0 else nc.scalar
            eng.dma_start(out=outT[h][:, tsl], in_=o_)

        def emit_C_group_split(tb, xsum, h):
            """Very last C group, column-split with TWO half-bank PSUM
            tiles so half-1's copy+DMA overlap half-2's matmuls (a single
            tile serializes on Tile's per-tile read/write tracking)."""
            t0 = tb * TBLK
            hsl = slice(h * 128, (h + 1) * 128)
            HB = TBLK // 2
            for i, c0 in enumerate((0, HB)):
                psD = pspD.tile([128, HB], F32, name=f"psDs{i}", tag="pD")
                for f in range(NF):
                    nc.tensor.matmul(psD, w2s[f][:, hsl],
                                     xsum[f][:, c0:c0 + HB],
                                     start=(f == 0), stop=(f == NF - 1))
                o_ = outp.tile([128, HB], BF16, name="osbs", tag="osb")
                nc.scalar.copy(o_, psD)
                eng = nc.sync if i == 0 else nc.scalar
                eng.dma_start(out=outT[h][:, t0 + c0:t0 + c0 + HB], in_=o_)

        def emit_C(tb, xsum, last=False):
            for h in range(NH):
                if last and h == NH - 1:
                    emit_C_group_split(tb, xsum, h)
                else:
                    emit_C_group(tb, xsum, h, lastblk=last)

        # ---- software pipeline: next block's phase-A groups are woven
        # BETWEEN this block's phase-B iterations (PE executes in emission
        # order, so independent work must be emitted before gated work).
        # Phase B is Vector-throughput-bound (~2.2us of DVE chain per
        # f-iteration); each interposed A group gives the DVE ~7us of
        # matmul cover to drain its chain backlog, so the B-group PSUM
        # leaders never wait on bank release. ----
        xs, rws, m1, m3 = pre0
        base1, base3 = emit_A(xs)
        heldC = None   # xsum of block NT-2, its C woven into the last block
        for tb in range(NT):
            x2s = [[None] * NF for _ in range(K)]
            xsum = [None] * NF
            Bf = lambda k, f: emit_B_f(tb, k, f, base1, base3, rws, m1, m3,
                                       x2s, xsum)
            if tb + 1 < NT:
                xsn, rwsn, m1n, m3n = load_block_inputs(tb + 1)
                b1n, b3n = [None] * NF, [None] * NF
                A = lambda f: emit_A_group(xsn, f, b1n, b3n)
                Bf(0, 0); Bf(0, 1)
                Bf(0, 2); Bf(0, 3); A(0)
                Bf(0, 4); Bf(0, 5); A(1)
                Bf(0, 6); Bf(1, 0); A(2)
                Bf(1, 1); Bf(1, 2); A(3)
                Bf(1, 3); Bf(1, 4); emit_a2(tb, 0, x2s)
                Bf(1, 5); A(4); Bf(1, 6)
                emit_a2(tb, 1, x2s)
                A(5); A(6)
                xs, rws, m1, m3 = xsn, rwsn, m1n, m3n
                base1, base3 = b1n, b3n
                if tb == NT - 2:
                    heldC = xsum      # defer C(NT-2) into the last block
                else:
                    emit_C(tb, xsum)
            else:
                # last block has no next-A cover; weave the held-back
                # C(NT-2) groups among the k=0 iterations instead. All held
                # groups must be emitted before B(1,0) writes xsum (the
                # single-buffered xsum tags roll over to this block there).
                hq = list(range(NH))
                C2 = lambda n: [emit_C_group(tb - 1, heldC, hq.pop(0),
                                             lastblk=True)
                                for _ in range(n)]
                Bx = lambda k, f: emit_B_f(tb, k, f, base1, base3, rws, m1,
                                           m3, x2s, xsum, do_xsum=False,
                                           altps=True)
                Bx(0, 0); Bx(1, 0); C2(3)
                Bx(0, 1); Bx(1, 1); C2(2)
                Bx(0, 2); Bx(1, 2); C2(3)
                Bx(0, 3); Bx(1, 3); C2(2)
                Bx(0, 4); Bx(1, 4); C2(2)
                Bx(0, 5); Bx(1, 5); C2(2)
                Bx(0, 6); Bx(1, 6); C2(2)
                for f in range(NF):
                    emit_xsum(f, x2s, xsum)
                emit_a2(tb, 0, x2s)
                emit_a2(tb, 1, x2s)
                emit_C(tb, xsum, last=True)

        if loop_cm is not None:
            loop_cm.__exit__(None, None, None)

    nc.finalize()
    return nc


def _q8(a, scale):
    return np.clip(a * scale, -240.0, 240.0).astype(NPF8E4)


def prepare_inputs(hidden_states, Wg, W1, W2, W3, A1, B1, A2, B2, A3, B3):
    """Host preprocessing: routing + per-core weight slicing/casting."""
    hidden_states, Wg, W1, W2, W3, A1, B1, A2, B2, A3, B3 = (
        np.asarray(a, dtype=np.float32)
        for a in (hidden_states, Wg, W1, W2, W3, A1, B1, A2, B2, A3, B3))
    x = np.ascontiguousarray(hidden_states.reshape(T, H))

    logits = x @ Wg.T.astype(np.float32)
    m = logits.max(-1, keepdims=True)
    p = np.exp(logits - m, dtype=np.float32)
    p /= p.sum(-1, keepdims=True)
    sel = np.argsort(-p, axis=-1, kind="stable")[:, :K]      # [T, K]
    rw = np.take_along_axis(p, sel, axis=1)
    rw = (rw / rw.sum(-1, keepdims=True)).astype(np.float32)  # [T, K]

    # Sort tokens by slot-0 EXPERT (block composition), then inside each
    # block by (slot0-chunk, slot1-chunk). Each block then decomposes into
    # a few contiguous column segments per slot, each touching a single
    # er-chunk — the device contracts every token column against only ITS
    # chunk instead of both.
    GE = E // NER                         # experts per er-chunk
    perm = np.argsort(sel[:, 0], kind="stable")
    for b in range(NT):
        idx = perm[b * TBLK:(b + 1) * TBLK]
        key = (sel[idx, 0] // GE) * NER + (sel[idx, 1] // GE)
        perm[b * TBLK:(b + 1) * TBLK] = idx[np.argsort(key, kind="stable")]
    x = np.ascontiguousarray(x[perm])
    sel = sel[perm]
    rw = np.ascontiguousarray(rw[perm])

    spec = []
    for b in range(NT):
        per_slot = []
        for k in range(K):
            ch = sel[b * TBLK:(b + 1) * TBLK, k] // GE
            segs, start = [], 0
            for i in range(1, TBLK + 1):
                if i == TBLK or ch[i] != ch[i - 1]:
                    segs.append((int(ch[start]), start, i))
                    start = i
            per_slot.append(tuple(segs))
        spec.append(tuple(per_slot))
    spec = tuple(spec)

    xT_np = np.ascontiguousarray(
        x.T.reshape(NH // 2, 2, 128, T).transpose(0, 2, 1, 3)
    ).astype(NPBF16)                                  # [NH//2, 128, 2, T]

    # per-slot one-hot masks over the (e, r) axis, transposed to [ER, T];
    # applied HOST-side to the returned a2 (masking is elementwise, so it
    # commutes with the cross-core partial sum)
    masks = np.zeros((K, ER, T), dtype=np.float32)
    for k in range(K):
        onehot = np.zeros((T, E), np.float32)
        onehot[np.arange(T), sel[:, k]] = 1.0
        masks[k] = np.repeat(onehot, R, axis=1).T
    # rw is pre-divided by SCALE: the device's x3 path multiplies the
    # SCALE-scaled (base3 + lora3) PSUM values by it, landing on true scale.
    rwr_np = np.ascontiguousarray(rw.T / SCALE).reshape(K, 1, T).astype(NPBF16)

    # flattened LoRA tensors (full copies; small)
    A1f = A1.reshape(ER, H)                      # [er, H]
    A3f = A3.reshape(ER, H)
    B2f = B2.transpose(0, 2, 1).reshape(ER, H)   # [er, H]

    # per-slot masked LoRA down-projections, computed host-side in fp32,
    # quantized to fp8e4 (x S_M) in the DoubleRow [Ki=128, Ko=NER, T] layout
    a1_all = x @ A1f.T.astype(np.float32)        # [T, ER]
    a3_all = x @ A3f.T.astype(np.float32)
    m1t_np = np.zeros((K, ER, T), dtype=NPF8E4)
    m3t_np = np.zeros((K, ER, T), dtype=NPF8E4)
    for k in range(K):
        mx = np.repeat(
            np.eye(E, dtype=np.float32)[sel[:, k]], R, axis=1)   # [T, ER]
        m1t_np[k] = _q8((a1_all * mx).T, S_M)
        m3t_np[k] = _q8((a3_all * mx).T, S_M)
    m1t_np = np.ascontiguousarray(
        m1t_np.reshape(K, NER, 128, T).transpose(0, 2, 1, 3))
    m3t_np = np.ascontiguousarray(
        m3t_np.reshape(K, NER, 128, T).transpose(0, 2, 1, 3))

    def pack_fmajor(wT):
        # [FS, H] -> [NF, 128, NH, 128]: [f, k, hc, m] = W[f*128+m, hc*128+k]
        return np.ascontiguousarray(
            wT.reshape(NF, 128, NH, 128).transpose(0, 3, 2, 1))

    in_maps = []
    for c in range(NCORES):
        fs = slice(c * FS, (c + 1) * FS)
        w1t_np = pack_fmajor(W1[fs]).astype(NPBF16)
        w3t_np = pack_fmajor(W3[fs]).astype(NPBF16)
        w2T = np.ascontiguousarray(W2[:, fs].T).astype(NPBF16)  # [FS, H]
        w2t_np = w2T.reshape(NF, 128, H)
        b1f = B1[:, fs, :].transpose(0, 2, 1).reshape(ER, FS)   # [er, f]
        b3f = B3[:, fs, :].transpose(0, 2, 1).reshape(ER, FS)
        b1t_np = np.ascontiguousarray(
            _q8(b1f, S_B).reshape(NER, 128, FS).transpose(1, 0, 2))
        b3t_np = np.ascontiguousarray(
            _q8(b3f, S_B).reshape(NER, 128, FS).transpose(1, 0, 2))
        a2f = A2[:, :, fs].reshape(ER, FS)                      # [er, f]
        a2t_np = np.ascontiguousarray(a2f.T).astype(NPBF16).reshape(NF, 128, ER)

        in_maps.append({
            "xT": xT_np, "w1t": w1t_np, "w3t": w3t_np, "w2t": w2t_np,
            "m1t": m1t_np, "m3t": m3t_np, "b1t": b1t_np, "b3t": b3t_np,
            "a2t": a2t_np,
            "rwr": rwr_np,
        })
    return in_maps, (B2f.astype(np.float32), masks, perm, spec)


_CACHED_NC = {}


def kernel(hidden_states, Wg, W1, W2, W3, A1, B1, A2, B2, A3, B3,
           _trace=False, _tmpdir=None):
    in_maps, (B2f, masks, perm, spec) = prepare_inputs(
        hidden_states, Wg, W1, W2, W3, A1, B1, A2, B2, A3, B3)
    if spec not in _CACHED_NC:
        _CACHED_NC[spec] = build_nc(spec=spec)
    nc = _CACHED_NC[spec]
    res = run_bass_kernel_spmd(nc, in_maps, list(range(NCORES)),
                               trace=_trace, tmpdir=_tmpdir)
    acc = np.zeros((NH, 128, T), np.float32)
    m2sum = np.zeros((K, ER, T), np.float32)
    for c in range(NCORES):
        acc += res.results[c]["outT"].astype(np.float32)
        m2sum += res.results[c]["m2o"].reshape(K, ER, T).astype(np.float32)
    out = acc.reshape(H, T)
    # host-side lora2: mask the (unmasked, core-summed) a2, then the final
    # LoRA up-projection is linear -> one small GEMM per slot
    for k in range(K):
        out += B2f.T @ (m2sum[k] * masks[k])
    outT_tok = out.T                       # [T, H], token-permuted order
    final = np.empty_like(outT_tok)
    final[perm] = outT_tok                 # undo the expert sort
    out = final.reshape(B, S, H)
    kernel.last_results = res
    return out


if __name__ == "__main__":
    nc = build_nc(spec=None)
    print("built ok")
